# revision 28
# baseline (speedup 1.0000x reference)
"""Two-layer GCN (GraphConv norm='both') on 8 Trainium2 NeuronCores.

Strategy (graph/data parallel per the sharding hint):
  - dst nodes partitioned across 8 cores (6250 rows each); weights replicated.
  - The dense weight is COMMUTED through the segment-sum:
    sum_e ns[s]*nd[d]*(x[s] @ W) = (sum_e ns[s]*nd[d]*x[s]) @ W, so layer 1
    aggregates RAW x rows. Each core receives the full x as a plain input
    split into two HBM tables (no phase-A compute, no layer-1 collective:
    gathers start immediately). Layer 2 aggregates relu rows r = relu(agg1
    @ W1 + b1) and applies W2 + b2 after its segment-sum; only the r table
    needs the halo-exchange AllGather (one per layer boundary).
  - Edge aggregation per core: edges grouped by (dst-tile, table-half) and
    sorted by source; source rows fetched with SWDGE dma_gather (256B f16
    rows, 8-chunk single-packet windows rotating over 4 SWDGE queues);
    segment-sum on the PE as one-hot matmuls into PSUM [f_in x dst]. The
    edge-weight selector S (carrying ns*nd) is built ON-CHIP on the DVE:
    S = (iota == dstcol) * nrm from a tiny per-chunk metadata table.
  - int16 gather indices cover only 32768 rows, so tables are split at row
    32768 (part a = each core's rows 0..4095 concatenated, part b = the
    rest). 256B single-row gathers have zero fetch waste, and part-a
    gathers depend only on the part-a AllGather (Tile tracks DRAM deps per
    tensor), letting next-layer gathers start while part b is in flight.
  - Biases are rank-1 matmul updates (ones-column x bias-row) into the
    same PSUM tile as the dense matmul; per-tile flow is
    pa=[f,d] -> f16 -> @W -> [d,f] node-major (+bias) -> relu/copy.

All feature math runs on-device in fp16 (f32 PSUM accumulation); index
preprocessing (edge partitioning/sorting, degree counts, gather schedule)
is host-side sharding prep.
"""

import numpy as np

N_NODES = 50000
N_EDGES = 600000
D = 128
N_CORES = 8
NPC = N_NODES // N_CORES          # 6250 nodes per core
NT = (NPC + 127) // 128           # 49 dst tiles per core
RS = 4096                         # part-a rows per core (8*RS = 32768)
NA = N_CORES * RS                 # 32768 rows in table a
NB = N_NODES - NA                 # 17232 rows in table b
W = 8                             # gather window size (chunks per call)

_CACHE = {}


def _host_prep(x, src, dst, W1, b1, W2, b2):
    x = np.asarray(x, dtype=np.float32)
    src = np.asarray(src, dtype=np.int64)
    dst = np.asarray(dst, dtype=np.int64)
    W1 = np.asarray(W1, dtype=np.float32)
    W2 = np.asarray(W2, dtype=np.float32)
    b1 = np.asarray(b1, dtype=np.float32)
    b2 = np.asarray(b2, dtype=np.float32)

    deg_out = np.bincount(src, minlength=N_NODES).astype(np.float32)
    deg_in = np.bincount(dst, minlength=N_NODES).astype(np.float32)
    norm_src = np.where(deg_out > 0, 1.0 / np.sqrt(np.maximum(deg_out, 1.0)), 0.0)
    norm_dst = np.where(deg_in > 0, 1.0 / np.sqrt(np.maximum(deg_in, 1.0)), 0.0)
    norm_src = norm_src.astype(np.float32)
    norm_dst = norm_dst.astype(np.float32)

    # Map node -> row in the split-table layout: part a = each core's rows
    # [0, RS) concatenated; part b = rows [RS, NPC) concatenated.
    nodes = np.arange(N_NODES, dtype=np.int64)
    ksn, rsn = nodes // NPC, nodes % NPC
    cat_of_node = np.where(rsn < RS, ksn * RS + rsn,
                           NA + ksn * (NPC - RS) + (rsn - RS))
    xcat = np.empty((N_NODES, D), dtype=np.float16)
    xcat[cat_of_node] = x.astype(np.float16)
    Xa, Xb = np.ascontiguousarray(xcat[:NA]), np.ascontiguousarray(xcat[NA:])

    cat = cat_of_node[src]
    half = (cat >= NA).astype(np.int64)

    # --- per-core edge grouping by (dst tile, half), sorted by cat ---
    per_core = []
    cnts = np.zeros((N_CORES, NT, 2), dtype=np.int64)
    for k in range(N_CORES):
        m = (dst >= k * NPC) & (dst < (k + 1) * NPC)
        c_k = cat[m]
        s_k = src[m]
        dl_k = dst[m] - k * NPC
        t_k = dl_k >> 7
        h_k = half[m]
        key = t_k * 2 + h_k
        order = np.lexsort((c_k, key))
        per_core.append((c_k[order], s_k[order], dl_k[order], key[order],
                         h_k[order]))
        cnts[k] = np.bincount(key, minlength=NT * 2).reshape(NT, 2)

    # shared static schedule: chunks per (tile, half), max over cores
    C_lo = np.maximum.reduce([(cnts[k, :, 0] + 127) // 128 for k in range(N_CORES)])
    C_hi = np.maximum.reduce([(cnts[k, :, 1] + 127) // 128 for k in range(N_CORES)])
    C_lo = np.where((C_lo + C_hi) == 0, 1, C_lo)
    base_lo = np.concatenate([[0], np.cumsum(C_lo)[:-1]])
    base_hi = np.concatenate([[0], np.cumsum(C_hi)[:-1]])
    n_lo, n_hi = int(C_lo.sum()), int(C_hi.sum())
    cons_base = np.concatenate([[0], np.cumsum(C_lo + C_hi)[:-1]])
    nch = n_lo + n_hi

    in_maps = []
    for k in range(N_CORES):
        c_k, s_k, dl_k, key, h_k = per_core[k]
        t_k = key >> 1
        grp_counts = np.bincount(key, minlength=NT * 2)
        grp_start = np.concatenate([[0], np.cumsum(grp_counts)[:-1]])
        rank = np.arange(len(key)) - grp_start[key]
        chunk_in_grp = rank >> 7
        lo_m = h_k == 0
        pos = np.where(lo_m, base_lo[t_k] * 128, base_hi[t_k] * 128) + rank

        idx_lo = np.zeros(n_lo * 128, dtype=np.int16)
        idx_hi = np.zeros(n_hi * 128, dtype=np.int16)
        idx_lo[pos[lo_m]] = c_k[lo_m].astype(np.int16)
        idx_hi[pos[~lo_m]] = (c_k[~lo_m] - NA).astype(np.int16)

        # consumption order per tile: lo chunks then hi chunks
        col = cons_base[t_k] + np.where(lo_m, chunk_in_grp,
                                        C_lo[t_k] + chunk_in_grp)
        row = rank & 127
        S = np.zeros((128, nch, 128), dtype=np.float16)
        S[row, col, dl_k & 127] = (norm_src[s_k]
                                   * norm_dst[dl_k + k * NPC]).astype(
                                       np.float16)

        in_maps.append(
            {
                "Xa": Xa,
                "Xb": Xb,
                "W1f": W1.astype(np.float16),
                "W2f": W2.astype(np.float16),
                "b1r": b1.reshape(1, 128).astype(np.float16),
                "b2r": b2.reshape(1, 128).astype(np.float16),
                "ones": np.ones((1, 128), dtype=np.float16),
                "idx_lo": np.tile(idx_lo.reshape(-1, 16).T, (8, 1)),
                "idx_hi": np.tile(idx_hi.reshape(-1, 16).T, (8, 1)),
                "S_all": np.ascontiguousarray(S.reshape(128, nch * 128)),
            }
        )
    sched = (tuple(int(v) for v in C_lo), tuple(int(v) for v in C_hi))
    return in_maps, sched, nch


def _build_program(sched, nch):
    import concourse.bacc as bacc
    import concourse.mybir as mybir
    import concourse.tile as tile
    from concourse.library_config import mlp

    C_lo, C_hi = sched
    base_lo = np.concatenate([[0], np.cumsum(C_lo)[:-1]]).astype(int)
    base_hi = np.concatenate([[0], np.cumsum(C_hi)[:-1]]).astype(int)
    n_lo, n_hi = int(sum(C_lo)), int(sum(C_hi))
    cons_base = np.concatenate(
        [[0], np.cumsum(np.array(C_lo) + np.array(C_hi))[:-1]]
    ).astype(int)
    ncp = (nch + W - 1) // W * W
    f16 = mybir.dt.float16
    f32 = mybir.dt.float32
    AF = mybir.ActivationFunctionType
    ALU = mybir.AluOpType

    nc = bacc.Bacc("TRN2", target_bir_lowering=False, debug=False,
                   num_devices=N_CORES, num_swdge_queues=4)

    Xa_d = nc.dram_tensor("Xa", [NA, D], f16, kind="ExternalInput")
    Xb_d = nc.dram_tensor("Xb", [NB, D], f16, kind="ExternalInput")
    W1_d = nc.dram_tensor("W1f", [128, 128], f16, kind="ExternalInput")
    W2_d = nc.dram_tensor("W2f", [128, 128], f16, kind="ExternalInput")
    b1_d = nc.dram_tensor("b1r", [1, 128], f16, kind="ExternalInput")
    b2_d = nc.dram_tensor("b2r", [1, 128], f16, kind="ExternalInput")
    on_d = nc.dram_tensor("ones", [1, 128], f16, kind="ExternalInput")
    ilo_d = nc.dram_tensor("idx_lo", [128, n_lo * 8], mybir.dt.int16,
                           kind="ExternalInput")
    ihi_d = nc.dram_tensor("idx_hi", [128, n_hi * 8], mybir.dt.int16,
                           kind="ExternalInput")
    S_d = nc.dram_tensor("S_all", [128, nch * 128], f16, kind="ExternalInput")

    rparts = (
        nc.dram_tensor("r_a", [RS, D], f16, kind="Internal"),
        nc.dram_tensor("r_b", [NPC - RS, D], f16, kind="Internal"),
    )
    Ra = nc.dram_tensor("Ra", [NA, D], f16, kind="Internal",
                        addr_space="Shared")
    Rb = nc.dram_tensor("Rb", [NB, D], f16, kind="Internal",
                        addr_space="Shared")
    out_d = nc.dram_tensor("outN", [NT * 128, D], f32, kind="ExternalOutput")

    qctr = [0]

    def next_q():
        q = qctr[0] % 4
        qctr[0] += 1
        return q

    with tile.TileContext(nc) as tc:
        with (
            tc.tile_pool(name="consts", bufs=1) as consts,
            tc.tile_pool(name="mt", bufs=16) as mt_pool,
            tc.tile_pool(name="st", bufs=12) as st_pool,
            tc.tile_pool(name="hb", bufs=6) as hb_pool,
            tc.tile_pool(name="pf", bufs=4) as pf_pool,
            tc.tile_pool(name="psa", bufs=3, space="PSUM") as psa_pool,
            tc.tile_pool(name="ps", bufs=5, space="PSUM") as ps_pool,
        ):
            nc.gpsimd.load_library(mlp)

            W1f = consts.tile([128, 128], f16, tag="W1f")
            W2f = consts.tile([128, 128], f16, tag="W2f")
            b1r = consts.tile([1, 128], f16, tag="b1r")
            b2r = consts.tile([1, 128], f16, tag="b2r")
            ones = consts.tile([1, 128], f16, tag="ones")
            idx_lo = consts.tile([128, n_lo * 8], mybir.dt.int16, tag="ilo")
            idx_hi = consts.tile([128, n_hi * 8], mybir.dt.int16, tag="ihi")
            nc.sync.dma_start(idx_lo[:], ilo_d.ap())
            nc.sync.dma_start(idx_hi[:], ihi_d.ap())
            nc.sync.dma_start(W1f[:], W1_d.ap())
            nc.sync.dma_start(W2f[:], W2_d.ap())
            nc.sync.dma_start(b1r[:], b1_d.ap())
            nc.sync.dma_start(b2r[:], b2_d.ap())
            nc.sync.dma_start(ones[:], on_d.ap())

            BT = 4
            TSPLIT = RS // 128      # 32

            def make_h_writer(h_dram, t_lo, t_hi):
                """Write tiles [t_lo, t_hi) into h_dram (rows rebased)."""
                nfull = min(t_hi, NPC // 128) - t_lo
                h3 = h_dram.ap()[0 : nfull * 128, :].rearrange(
                    "(a p) d -> p a d", p=128
                )
                state = {}

                def write(t, produce):
                    tl_ = t - t_lo
                    if tl_ < nfull:
                        g = tl_ - tl_ % BT
                        if tl_ % BT == 0:
                            state["buf"] = hb_pool.tile(
                                [128, BT, 128], f16, tag="hstage", name="hstage"
                            )
                        produce(state["buf"][:, tl_ % BT, :])
                        if tl_ % BT == BT - 1 or tl_ == nfull - 1:
                            n = tl_ - g + 1
                            nc.sync.dma_start(h3[:, g : g + n, :],
                                              state["buf"][:, 0:n, :])
                    else:
                        rows = NPC - t * 128
                        tl = hb_pool.tile([128, 128], f16, tag="hrag",
                                          name="hrag")
                        produce(tl[:])
                        nc.sync.dma_start(
                            h_dram.ap()[tl_ * 128 : tl_ * 128 + rows, :],
                            tl[:rows, :],
                        )

                return write

            def sub_allgather(h_sub, H_out):
                nc.gpsimd.collective_compute(
                    "AllGather", mybir.AluOpType.bypass,
                    replica_groups=[list(range(N_CORES))],
                    ins=[h_sub.ap()], outs=[H_out.ap()],
                )

            def agg_phase(Ta, Tb, out_cb):
                """Gather in fixed W-chunk windows per (lo, hi) stream; per
                dst tile accumulate segment-sum matmuls into psum [f x d],
                then out_cb(t, pa). Selector built on DVE per S-window."""
                streams = {"lo": (Ta, idx_lo, n_lo), "hi": (Tb, idx_hi, n_hi)}
                mt_tiles = {}
                st_tiles = {}

                def ensure_window(s, w):
                    key = (s, w)
                    if key in mt_tiles:
                        return
                    tab, idx_t, n_s = streams[s]
                    cb = w * W
                    cw = min(W, n_s - cb)
                    mt = mt_pool.tile([128, cw, 128], f16, tag="mt")
                    nc.gpsimd.dma_gather(
                        mt[:], tab.ap(),
                        idx_t[:, cb * 8 : (cb + cw) * 8],
                        cw * 128, cw * 128, 128,
                        queue_num=next_q(),
                    )
                    mt_tiles[key] = mt

                def ensure_s(cons):
                    sw, so = cons // W, cons % W
                    if sw not in st_tiles:
                        cb = sw * W
                        cw = min(W, nch - cb)
                        st = st_pool.tile([128, cw * 128], f16, tag="st")
                        nc.scalar.dma_start(
                            st[:], S_d.ap()[:, cb * 128 : (cb + cw) * 128])
                        st_tiles[sw] = st
                    return st_tiles[sw][:, so * 128 : (so + 1) * 128]

                # Prefetch lo windows so the in-order Pool engine has queued
                # DMA work to drain while the first hi window waits on the
                # part-b table.
                for w in range(min(14, (n_lo + W - 1) // W)):
                    ensure_window("lo", w)

                for t in range(NT):
                    nlo, nhi = C_lo[t], C_hi[t]
                    pa = ps_pool.tile([128, 128], f32, tag="pa")
                    for i in range(nlo + nhi):
                        if i < nlo:
                            s, sc = "lo", int(base_lo[t]) + i
                        else:
                            s, sc = "hi", int(base_hi[t]) + (i - nlo)
                        w, o = sc // W, sc % W
                        ensure_window(s, w)
                        cons = int(cons_base[t]) + i
                        s_ap = ensure_s(cons)
                        nc.tensor.matmul(
                            pa[:], mt_tiles[(s, w)][:, o, :], s_ap,
                            start=(i == 0), stop=(i == nlo + nhi - 1),
                        )
                    out_cb(t, pa)

            # ---- layer 1: agg(x) -> @W1 + b1 -> relu -> r parts + AG ----
            w1_a = make_h_writer(rparts[0], 0, TSPLIT)
            w1_b = make_h_writer(rparts[1], TSPLIT, NT)

            def phase_1(t, pa):
                paf = pf_pool.tile([128, 128], f16, tag="paf")
                nc.scalar.activation(paf[:], pa[:], AF.Copy)
                ph = psa_pool.tile([128, 128], f32, tag="ph", name="ph")
                nc.tensor.matmul(ph[:], paf[:], W1f[:], start=True, stop=False)
                nc.tensor.matmul(ph[:], ones[:], b1r[:], start=False, stop=True)
                (w1_a if t < TSPLIT else w1_b)(
                    t, lambda dst, ph=ph: nc.scalar.activation(
                        dst, ph[:], AF.Relu))
                if t == TSPLIT - 1:
                    sub_allgather(rparts[0], Ra)

            agg_phase(Xa_d, Xb_d, phase_1)
            sub_allgather(rparts[1], Rb)

            # ---- layer 2: agg(r) -> @W2 + b2 -> out (node-major) ----
            o3 = out_d.ap().rearrange("(a p) d -> p a d", p=128)
            ostate = {}

            def phase_2(t, pa):
                paf = pf_pool.tile([128, 128], f16, tag="paf")
                nc.scalar.activation(paf[:], pa[:], AF.Copy)
                ph = psa_pool.tile([128, 128], f32, tag="ph", name="ph2")
                nc.tensor.matmul(ph[:], paf[:], W2f[:], start=True, stop=False)
                nc.tensor.matmul(ph[:], ones[:], b2r[:], start=False, stop=True)
                g = t - t % BT
                if t % BT == 0:
                    ostate["buf"] = hb_pool.tile([128, BT, 128], f32,
                                                 tag="ostage", name="ostage")
                nc.scalar.activation(ostate["buf"][:, t % BT, :], ph[:],
                                     AF.Copy)
                if t % BT == BT - 1 or t == NT - 1:
                    n = t - g + 1
                    nc.sync.dma_start(o3[:, g : g + n, :],
                                      ostate["buf"][:, 0:n, :])

            agg_phase(Ra, Rb, phase_2)

    nc.compile()
    return nc


def kernel(x, src, dst, W1, b1, W2, b2):
    from concourse.bass_utils import run_bass_kernel_spmd

    in_maps, sched, nch = _host_prep(x, src, dst, W1, b1, W2, b2)
    key = (sched, nch)
    if key not in _CACHE:
        _CACHE[key] = _build_program(sched, nch)
    nc = _CACHE[key]
    res = run_bass_kernel_spmd(nc, in_maps, core_ids=list(range(N_CORES)))
    out = np.empty((N_NODES, D), dtype=np.float32)
    for k in range(N_CORES):
        out[k * NPC : (k + 1) * NPC] = res.results[k]["outN"][:NPC]
    return out


# revision 34
# speedup vs baseline: 1.0319x; 1.0319x over previous
"""Two-layer GCN (GraphConv norm='both') on 8 Trainium2 NeuronCores.

Strategy (graph/data parallel per the sharding hint):
  - dst nodes partitioned across 8 cores (6250 rows each); weights replicated.
  - The dense weight is COMMUTED through the segment-sum:
    sum_e ns[s]*nd[d]*(x[s] @ W) = (sum_e ns[s]*nd[d]*x[s]) @ W, so layer 1
    aggregates RAW x rows. Each core receives the full x as a plain input
    split into two HBM tables (no phase-A compute, no layer-1 collective:
    gathers start immediately). Layer 2 aggregates relu rows r = relu(agg1
    @ W1 + b1) and applies W2 + b2 after its segment-sum; only the r table
    needs the halo-exchange AllGather (one per layer boundary).
  - Edge aggregation per core: edges grouped by (dst-tile, table-half) and
    sorted by source; source rows fetched with SWDGE dma_gather (256B f16
    rows, 8-chunk single-packet windows rotating over 4 SWDGE queues);
    segment-sum on the PE as one-hot matmuls into PSUM [f_in x dst]. The
    edge-weight selector S (carrying ns*nd) is built ON-CHIP on the DVE:
    S = (iota == dstcol) * nrm from a tiny per-chunk metadata table.
  - int16 gather indices cover only 32768 rows, so tables are split at row
    32768 (part a = each core's rows 0..4095 concatenated, part b = the
    rest). 256B single-row gathers have zero fetch waste, and part-a
    gathers depend only on the part-a AllGather (Tile tracks DRAM deps per
    tensor), letting next-layer gathers start while part b is in flight.
  - Biases are rank-1 matmul updates (ones-column x bias-row) into the
    same PSUM tile as the dense matmul; per-tile flow is
    pa=[f,d] -> f16 -> @W -> [d,f] node-major (+bias) -> relu/copy.

All feature math runs on-device in fp16 (f32 PSUM accumulation); index
preprocessing (edge partitioning/sorting, degree counts, gather schedule)
is host-side sharding prep.
"""

import numpy as np

N_NODES = 50000
N_EDGES = 600000
D = 128
N_CORES = 8
NPC = N_NODES // N_CORES          # 6250 nodes per core
NT = (NPC + 127) // 128           # 49 dst tiles per core
RS = 4096                         # part-a rows per core (8*RS = 32768)
NA = N_CORES * RS                 # 32768 rows in table a
NB = N_NODES - NA                 # 17232 rows in table b
W = 8                             # gather window size (chunks per call)

_CACHE = {}


def _host_prep(x, src, dst, W1, b1, W2, b2):
    x = np.asarray(x, dtype=np.float32)
    src = np.asarray(src, dtype=np.int64)
    dst = np.asarray(dst, dtype=np.int64)
    W1 = np.asarray(W1, dtype=np.float32)
    W2 = np.asarray(W2, dtype=np.float32)
    b1 = np.asarray(b1, dtype=np.float32)
    b2 = np.asarray(b2, dtype=np.float32)

    deg_out = np.bincount(src, minlength=N_NODES).astype(np.float32)
    deg_in = np.bincount(dst, minlength=N_NODES).astype(np.float32)
    norm_src = np.where(deg_out > 0, 1.0 / np.sqrt(np.maximum(deg_out, 1.0)), 0.0)
    norm_dst = np.where(deg_in > 0, 1.0 / np.sqrt(np.maximum(deg_in, 1.0)), 0.0)
    norm_src = norm_src.astype(np.float32)
    norm_dst = norm_dst.astype(np.float32)

    # Map node -> row in the split-table layout: part a = each core's rows
    # [0, RS) concatenated; part b = rows [RS, NPC) concatenated.
    nodes = np.arange(N_NODES, dtype=np.int64)
    ksn, rsn = nodes // NPC, nodes % NPC
    cat_of_node = np.where(rsn < RS, ksn * RS + rsn,
                           NA + ksn * (NPC - RS) + (rsn - RS))
    xcat = np.empty((N_NODES, D), dtype=np.float16)
    xcat[cat_of_node] = x.astype(np.float16)
    Xa, Xb = np.ascontiguousarray(xcat[:NA]), np.ascontiguousarray(xcat[NA:])

    cat = cat_of_node[src]
    half = (cat >= NA).astype(np.int64)

    # --- per-core edge grouping by (dst tile, half), sorted by cat ---
    per_core = []
    cnts = np.zeros((N_CORES, NT, 2), dtype=np.int64)
    for k in range(N_CORES):
        m = (dst >= k * NPC) & (dst < (k + 1) * NPC)
        c_k = cat[m]
        s_k = src[m]
        dl_k = dst[m] - k * NPC
        t_k = dl_k >> 7
        h_k = half[m]
        key = t_k * 2 + h_k
        order = np.lexsort((c_k, key))
        per_core.append((c_k[order], s_k[order], dl_k[order], key[order],
                         h_k[order]))
        cnts[k] = np.bincount(key, minlength=NT * 2).reshape(NT, 2)

    # shared static schedule: chunks per (tile, half), max over cores
    C_lo = np.maximum.reduce([(cnts[k, :, 0] + 127) // 128 for k in range(N_CORES)])
    C_hi = np.maximum.reduce([(cnts[k, :, 1] + 127) // 128 for k in range(N_CORES)])
    C_lo = np.where((C_lo + C_hi) == 0, 1, C_lo)
    base_lo = np.concatenate([[0], np.cumsum(C_lo)[:-1]])
    base_hi = np.concatenate([[0], np.cumsum(C_hi)[:-1]])
    n_lo, n_hi = int(C_lo.sum()), int(C_hi.sum())
    cons_base = np.concatenate([[0], np.cumsum(C_lo + C_hi)[:-1]])
    nch = n_lo + n_hi

    in_maps = []
    for k in range(N_CORES):
        c_k, s_k, dl_k, key, h_k = per_core[k]
        t_k = key >> 1
        grp_counts = np.bincount(key, minlength=NT * 2)
        grp_start = np.concatenate([[0], np.cumsum(grp_counts)[:-1]])
        rank = np.arange(len(key)) - grp_start[key]
        chunk_in_grp = rank >> 7
        lo_m = h_k == 0
        pos = np.where(lo_m, base_lo[t_k] * 128, base_hi[t_k] * 128) + rank

        idx_lo = np.zeros(n_lo * 128, dtype=np.int16)
        idx_hi = np.zeros(n_hi * 128, dtype=np.int16)
        idx_lo[pos[lo_m]] = c_k[lo_m].astype(np.int16)
        idx_hi[pos[~lo_m]] = (c_k[~lo_m] - NA).astype(np.int16)

        # consumption order per tile: lo chunks then hi chunks
        col = cons_base[t_k] + np.where(lo_m, chunk_in_grp,
                                        C_lo[t_k] + chunk_in_grp)
        row = rank & 127
        S = np.zeros((128, nch, 128), dtype=np.float16)
        S[row, col, dl_k & 127] = (norm_src[s_k]
                                   * norm_dst[dl_k + k * NPC]).astype(
                                       np.float16)

        in_maps.append(
            {
                "Xa": Xa,
                "Xb": Xb,
                "W1f": W1.astype(np.float16),
                "W2f": W2.astype(np.float16),
                "b1r": b1.reshape(1, 128).astype(np.float16),
                "b2r": b2.reshape(1, 128).astype(np.float16),
                "ones": np.ones((1, 128), dtype=np.float16),
                "idx_lo": np.tile(idx_lo.reshape(-1, 16).T, (8, 1)),
                "idx_hi": np.tile(idx_hi.reshape(-1, 16).T, (8, 1)),
                "S_all": np.ascontiguousarray(S.reshape(128, nch * 128)),
            }
        )
    sched = (tuple(int(v) for v in C_lo), tuple(int(v) for v in C_hi))
    return in_maps, sched, nch


def _build_program(sched, nch):
    import concourse.bacc as bacc
    import concourse.mybir as mybir
    import concourse.tile as tile
    from concourse.library_config import mlp

    C_lo, C_hi = sched
    base_lo = np.concatenate([[0], np.cumsum(C_lo)[:-1]]).astype(int)
    base_hi = np.concatenate([[0], np.cumsum(C_hi)[:-1]]).astype(int)
    n_lo, n_hi = int(sum(C_lo)), int(sum(C_hi))
    cons_base = np.concatenate(
        [[0], np.cumsum(np.array(C_lo) + np.array(C_hi))[:-1]]
    ).astype(int)
    ncp = (nch + W - 1) // W * W
    f16 = mybir.dt.float16
    f32 = mybir.dt.float32
    AF = mybir.ActivationFunctionType
    ALU = mybir.AluOpType

    nc = bacc.Bacc("TRN2", target_bir_lowering=False, debug=False,
                   num_devices=N_CORES, num_swdge_queues=4)

    Xa_d = nc.dram_tensor("Xa", [NA, D], f16, kind="ExternalInput")
    Xb_d = nc.dram_tensor("Xb", [NB, D], f16, kind="ExternalInput")
    W1_d = nc.dram_tensor("W1f", [128, 128], f16, kind="ExternalInput")
    W2_d = nc.dram_tensor("W2f", [128, 128], f16, kind="ExternalInput")
    b1_d = nc.dram_tensor("b1r", [1, 128], f16, kind="ExternalInput")
    b2_d = nc.dram_tensor("b2r", [1, 128], f16, kind="ExternalInput")
    on_d = nc.dram_tensor("ones", [1, 128], f16, kind="ExternalInput")
    ilo_d = nc.dram_tensor("idx_lo", [128, n_lo * 8], mybir.dt.int16,
                           kind="ExternalInput")
    ihi_d = nc.dram_tensor("idx_hi", [128, n_hi * 8], mybir.dt.int16,
                           kind="ExternalInput")
    S_d = nc.dram_tensor("S_all", [128, nch * 128], f16, kind="ExternalInput")

    rparts = (
        nc.dram_tensor("r_a", [RS, D], f16, kind="Internal"),
        nc.dram_tensor("r_b", [NPC - RS, D], f16, kind="Internal"),
    )
    Ra = nc.dram_tensor("Ra", [NA, D], f16, kind="Internal",
                        addr_space="Shared")
    Rb = nc.dram_tensor("Rb", [NB, D], f16, kind="Internal",
                        addr_space="Shared")
    out_d = nc.dram_tensor("outN", [NT * 128, D], f32, kind="ExternalOutput")

    qctr = [0]

    def next_q():
        q = qctr[0] % 4
        qctr[0] += 1
        return q

    with tile.TileContext(nc) as tc:
        with (
            tc.tile_pool(name="consts", bufs=1) as consts,
            tc.tile_pool(name="mt", bufs=20) as mt_pool,
            tc.tile_pool(name="st", bufs=20) as st_pool,
            tc.tile_pool(name="hb", bufs=6) as hb_pool,
            tc.tile_pool(name="pf", bufs=4) as pf_pool,
            tc.tile_pool(name="psa", bufs=3, space="PSUM") as psa_pool,
            tc.tile_pool(name="ps", bufs=5, space="PSUM") as ps_pool,
        ):
            nc.gpsimd.load_library(mlp)

            W1f = consts.tile([128, 128], f16, tag="W1f")
            W2f = consts.tile([128, 128], f16, tag="W2f")
            b1r = consts.tile([1, 128], f16, tag="b1r")
            b2r = consts.tile([1, 128], f16, tag="b2r")
            ones = consts.tile([1, 128], f16, tag="ones")
            idx_lo = consts.tile([128, n_lo * 8], mybir.dt.int16, tag="ilo")
            idx_hi = consts.tile([128, n_hi * 8], mybir.dt.int16, tag="ihi")
            nc.sync.dma_start(idx_lo[:], ilo_d.ap())
            nc.sync.dma_start(idx_hi[:], ihi_d.ap())
            nc.sync.dma_start(W1f[:], W1_d.ap())
            nc.sync.dma_start(W2f[:], W2_d.ap())
            nc.sync.dma_start(b1r[:], b1_d.ap())
            nc.sync.dma_start(b2r[:], b2_d.ap())
            nc.sync.dma_start(ones[:], on_d.ap())

            BT = 4
            TSPLIT = RS // 128      # 32

            def make_h_writer(h_dram, t_lo, t_hi):
                """Write tiles [t_lo, t_hi) into h_dram (rows rebased)."""
                nfull = min(t_hi, NPC // 128) - t_lo
                h3 = h_dram.ap()[0 : nfull * 128, :].rearrange(
                    "(a p) d -> p a d", p=128
                )
                state = {}

                def write(t, produce):
                    tl_ = t - t_lo
                    if tl_ < nfull:
                        g = tl_ - tl_ % BT
                        if tl_ % BT == 0:
                            state["buf"] = hb_pool.tile(
                                [128, BT, 128], f16, tag="hstage", name="hstage"
                            )
                        produce(state["buf"][:, tl_ % BT, :])
                        if tl_ % BT == BT - 1 or tl_ == nfull - 1:
                            n = tl_ - g + 1
                            nc.scalar.dma_start(h3[:, g : g + n, :],
                                                state["buf"][:, 0:n, :])
                    else:
                        rows = NPC - t * 128
                        tl = hb_pool.tile([128, 128], f16, tag="hrag",
                                          name="hrag")
                        produce(tl[:])
                        nc.scalar.dma_start(
                            h_dram.ap()[tl_ * 128 : tl_ * 128 + rows, :],
                            tl[:rows, :],
                        )

                return write

            def sub_allgather(h_sub, H_out):
                nc.gpsimd.collective_compute(
                    "AllGather", mybir.AluOpType.bypass,
                    replica_groups=[list(range(N_CORES))],
                    ins=[h_sub.ap()], outs=[H_out.ap()],
                )

            def agg_phase(Ta, Tb, out_cb):
                """Gather in fixed W-chunk windows per (lo, hi) stream; per
                dst tile accumulate segment-sum matmuls into psum [f x d],
                then out_cb(t, pa). Selector built on DVE per S-window."""
                streams = {"lo": (Ta, idx_lo, n_lo), "hi": (Tb, idx_hi, n_hi)}
                mt_tiles = {}
                st_tiles = {}

                def ensure_window(s, w):
                    key = (s, w)
                    if key in mt_tiles:
                        return
                    tab, idx_t, n_s = streams[s]
                    cb = w * W
                    cw = min(W, n_s - cb)
                    mt = mt_pool.tile([128, cw, 128], f16, tag="mt")
                    nc.gpsimd.dma_gather(
                        mt[:], tab.ap(),
                        idx_t[:, cb * 8 : (cb + cw) * 8],
                        cw * 128, cw * 128, 128,
                        queue_num=next_q(),
                    )
                    mt_tiles[key] = mt

                def ensure_s(cons):
                    sw, so = cons // W, cons % W
                    if sw not in st_tiles:
                        cb = sw * W
                        cw = min(W, nch - cb)
                        st = st_pool.tile([128, cw * 128], f16, tag="st")
                        nc.sync.dma_start(
                            st[:], S_d.ap()[:, cb * 128 : (cb + cw) * 128])
                        st_tiles[sw] = st
                    return st_tiles[sw][:, so * 128 : (so + 1) * 128]

                # Prefetch lo windows so the in-order Pool engine has queued
                # DMA work to drain while the first hi window waits on the
                # part-b table.
                for w in range(min(16, (n_lo + W - 1) // W)):
                    ensure_window("lo", w)

                for t in range(NT):
                    nlo, nhi = C_lo[t], C_hi[t]
                    pa = ps_pool.tile([128, 128], f32, tag="pa")
                    for i in range(nlo + nhi):
                        if i < nlo:
                            s, sc = "lo", int(base_lo[t]) + i
                        else:
                            s, sc = "hi", int(base_hi[t]) + (i - nlo)
                        w, o = sc // W, sc % W
                        ensure_window(s, w)
                        cons = int(cons_base[t]) + i
                        s_ap = ensure_s(cons)
                        nc.tensor.matmul(
                            pa[:], mt_tiles[(s, w)][:, o, :], s_ap,
                            start=(i == 0), stop=(i == nlo + nhi - 1),
                        )
                    out_cb(t, pa)

            # ---- layer 1: agg(x) -> @W1 + b1 -> relu -> r parts + AG ----
            w1_a = make_h_writer(rparts[0], 0, TSPLIT)
            w1_b = make_h_writer(rparts[1], TSPLIT, NT)

            def phase_1(t, pa):
                paf = pf_pool.tile([128, 128], f16, tag="paf")
                nc.scalar.activation(paf[:], pa[:], AF.Copy)
                ph = psa_pool.tile([128, 128], f32, tag="ph", name="ph")
                nc.tensor.matmul(ph[:], paf[:], W1f[:], start=True, stop=False)
                nc.tensor.matmul(ph[:], ones[:], b1r[:], start=False, stop=True)
                (w1_a if t < TSPLIT else w1_b)(
                    t, lambda dst, ph=ph: nc.scalar.activation(
                        dst, ph[:], AF.Relu))
                if t == TSPLIT - 1:
                    sub_allgather(rparts[0], Ra)

            agg_phase(Xa_d, Xb_d, phase_1)
            sub_allgather(rparts[1], Rb)

            # ---- layer 2: agg(r) -> @W2 + b2 -> out (node-major) ----
            o3 = out_d.ap().rearrange("(a p) d -> p a d", p=128)
            ostate = {}

            def phase_2(t, pa):
                paf = pf_pool.tile([128, 128], f16, tag="paf")
                nc.scalar.activation(paf[:], pa[:], AF.Copy)
                ph = psa_pool.tile([128, 128], f32, tag="ph", name="ph2")
                nc.tensor.matmul(ph[:], paf[:], W2f[:], start=True, stop=False)
                nc.tensor.matmul(ph[:], ones[:], b2r[:], start=False, stop=True)
                g = t - t % BT
                if t % BT == 0:
                    ostate["buf"] = hb_pool.tile([128, BT, 128], f32,
                                                 tag="ostage", name="ostage")
                nc.scalar.activation(ostate["buf"][:, t % BT, :], ph[:],
                                     AF.Copy)
                if t % BT == BT - 1 or t == NT - 1:
                    n = t - g + 1
                    nc.scalar.dma_start(o3[:, g : g + n, :],
                                        ostate["buf"][:, 0:n, :])

            agg_phase(Ra, Rb, phase_2)

    nc.compile()
    return nc


def kernel(x, src, dst, W1, b1, W2, b2):
    from concourse.bass_utils import run_bass_kernel_spmd

    in_maps, sched, nch = _host_prep(x, src, dst, W1, b1, W2, b2)
    key = (sched, nch)
    if key not in _CACHE:
        _CACHE[key] = _build_program(sched, nch)
    nc = _CACHE[key]
    res = run_bass_kernel_spmd(nc, in_maps, core_ids=list(range(N_CORES)))
    out = np.empty((N_NODES, D), dtype=np.float32)
    for k in range(N_CORES):
        out[k * NPC : (k + 1) * NPC] = res.results[k]["outN"][:NPC]
    return out


# revision 39
# speedup vs baseline: 1.0537x; 1.0210x over previous
"""Two-layer GCN (GraphConv norm='both') on 8 Trainium2 NeuronCores.

Strategy (graph/data parallel per the sharding hint):
  - dst nodes partitioned across 8 cores (6250 rows each); weights replicated.
  - The dense weight is COMMUTED through the segment-sum:
    sum_e ns[s]*nd[d]*(x[s] @ W) = (sum_e ns[s]*nd[d]*x[s]) @ W, so layer 1
    aggregates RAW x rows. Each core receives the full x as a plain input
    split into two HBM tables (no phase-A compute, no layer-1 collective:
    gathers start immediately). Layer 2 aggregates relu rows r = relu(agg1
    @ W1 + b1) and applies W2 + b2 after its segment-sum; only the r table
    needs the halo-exchange AllGather (one per layer boundary).
  - Edge aggregation per core: edges grouped by (dst-tile, table-half) and
    sorted by source; source rows fetched with SWDGE dma_gather (256B f16
    rows, 8-chunk single-packet windows rotating over 4 SWDGE queues);
    segment-sum on the PE as one-hot matmuls into PSUM [f_in x dst]. The
    edge-weight selector S (carrying ns*nd) is built ON-CHIP on the DVE:
    S = (iota == dstcol) * nrm from a tiny per-chunk metadata table.
  - int16 gather indices cover only 32768 rows, so tables are split at row
    32768 (part a = each core's rows 0..4095 concatenated, part b = the
    rest). 256B single-row gathers have zero fetch waste, and part-a
    gathers depend only on the part-a AllGather (Tile tracks DRAM deps per
    tensor), letting next-layer gathers start while part b is in flight.
  - Biases are rank-1 matmul updates (ones-column x bias-row) into the
    same PSUM tile as the dense matmul; per-tile flow is
    pa=[f,d] -> f16 -> @W -> [d,f] node-major (+bias) -> relu/copy.

All feature math runs on-device in fp16 (f32 PSUM accumulation); index
preprocessing (edge partitioning/sorting, degree counts, gather schedule)
is host-side sharding prep.
"""

import numpy as np

N_NODES = 50000
N_EDGES = 600000
D = 128
N_CORES = 8
NPC = N_NODES // N_CORES          # 6250 nodes per core
NT = (NPC + 127) // 128           # 49 dst tiles per core
RS = 4096                         # part-a rows per core (8*RS = 32768)
NA = N_CORES * RS                 # 32768 rows in table a
NB = N_NODES - NA                 # 17232 rows in table b
W = 8                             # gather window size (chunks per call)

_CACHE = {}


def _host_prep(x, src, dst, W1, b1, W2, b2):
    x = np.asarray(x, dtype=np.float32)
    src = np.asarray(src, dtype=np.int64)
    dst = np.asarray(dst, dtype=np.int64)
    W1 = np.asarray(W1, dtype=np.float32)
    W2 = np.asarray(W2, dtype=np.float32)
    b1 = np.asarray(b1, dtype=np.float32)
    b2 = np.asarray(b2, dtype=np.float32)

    deg_out = np.bincount(src, minlength=N_NODES).astype(np.float32)
    deg_in = np.bincount(dst, minlength=N_NODES).astype(np.float32)
    norm_src = np.where(deg_out > 0, 1.0 / np.sqrt(np.maximum(deg_out, 1.0)), 0.0)
    norm_dst = np.where(deg_in > 0, 1.0 / np.sqrt(np.maximum(deg_in, 1.0)), 0.0)
    norm_src = norm_src.astype(np.float32)
    norm_dst = norm_dst.astype(np.float32)

    # Map node -> row in the split-table layout: part a = each core's rows
    # [0, RS) concatenated; part b = rows [RS, NPC) concatenated.
    nodes = np.arange(N_NODES, dtype=np.int64)
    ksn, rsn = nodes // NPC, nodes % NPC
    cat_of_node = np.where(rsn < RS, ksn * RS + rsn,
                           NA + ksn * (NPC - RS) + (rsn - RS))
    xcat = np.empty((N_NODES, D), dtype=np.float16)
    xcat[cat_of_node] = x.astype(np.float16)
    Xa, Xb = np.ascontiguousarray(xcat[:NA]), np.ascontiguousarray(xcat[NA:])

    cat = cat_of_node[src]
    half = (cat >= NA).astype(np.int64)

    # --- per-core edge grouping by (dst tile, half), sorted by cat ---
    per_core = []
    cnts = np.zeros((N_CORES, NT, 2), dtype=np.int64)
    for k in range(N_CORES):
        m = (dst >= k * NPC) & (dst < (k + 1) * NPC)
        c_k = cat[m]
        s_k = src[m]
        dl_k = dst[m] - k * NPC
        t_k = dl_k >> 7
        h_k = half[m]
        key = t_k * 2 + h_k
        order = np.lexsort((c_k, key))
        per_core.append((c_k[order], s_k[order], dl_k[order], key[order],
                         h_k[order]))
        cnts[k] = np.bincount(key, minlength=NT * 2).reshape(NT, 2)

    # shared static schedule: chunks per (tile, half), max over cores
    C_lo = np.maximum.reduce([(cnts[k, :, 0] + 127) // 128 for k in range(N_CORES)])
    C_hi = np.maximum.reduce([(cnts[k, :, 1] + 127) // 128 for k in range(N_CORES)])
    C_lo = np.where((C_lo + C_hi) == 0, 1, C_lo)
    base_lo = np.concatenate([[0], np.cumsum(C_lo)[:-1]])
    base_hi = np.concatenate([[0], np.cumsum(C_hi)[:-1]])
    n_lo, n_hi = int(C_lo.sum()), int(C_hi.sum())
    cons_base = np.concatenate([[0], np.cumsum(C_lo + C_hi)[:-1]])
    nch = n_lo + n_hi

    in_maps = []
    for k in range(N_CORES):
        c_k, s_k, dl_k, key, h_k = per_core[k]
        t_k = key >> 1
        grp_counts = np.bincount(key, minlength=NT * 2)
        grp_start = np.concatenate([[0], np.cumsum(grp_counts)[:-1]])
        rank = np.arange(len(key)) - grp_start[key]
        chunk_in_grp = rank >> 7
        lo_m = h_k == 0
        pos = np.where(lo_m, base_lo[t_k] * 128, base_hi[t_k] * 128) + rank

        idx_lo = np.zeros(n_lo * 128, dtype=np.int16)
        idx_hi = np.zeros(n_hi * 128, dtype=np.int16)
        idx_lo[pos[lo_m]] = c_k[lo_m].astype(np.int16)
        idx_hi[pos[~lo_m]] = (c_k[~lo_m] - NA).astype(np.int16)

        # consumption order per tile: lo chunks then hi chunks
        col = cons_base[t_k] + np.where(lo_m, chunk_in_grp,
                                        C_lo[t_k] + chunk_in_grp)
        row = rank & 127
        meta_dst = np.full((128, nch), 999.0, dtype=np.float32)
        meta_nrm = np.zeros((128, nch), dtype=np.float32)
        meta_dst[row, col] = (dl_k & 127).astype(np.float32)
        meta_nrm[row, col] = (norm_src[s_k]
                              * norm_dst[dl_k + k * NPC]).astype(np.float32)

        in_maps.append(
            {
                "Xa": Xa,
                "Xb": Xb,
                "W1f": W1.astype(np.float16),
                "W2f": W2.astype(np.float16),
                "b1r": b1.reshape(1, 128).astype(np.float16),
                "b2r": b2.reshape(1, 128).astype(np.float16),
                "ones": np.ones((1, 128), dtype=np.float16),
                "iota": np.tile(np.arange(128, dtype=np.float16), (128, 1)),
                "idx_lo": np.tile(idx_lo.reshape(-1, 16).T, (8, 1)),
                "idx_hi": np.tile(idx_hi.reshape(-1, 16).T, (8, 1)),
                "meta_dst": meta_dst,
                "meta_nrm": meta_nrm,
            }
        )
    sched = (tuple(int(v) for v in C_lo), tuple(int(v) for v in C_hi))
    return in_maps, sched, nch


def _build_program(sched, nch):
    import concourse.bacc as bacc
    import concourse.mybir as mybir
    import concourse.tile as tile
    from concourse.library_config import mlp

    C_lo, C_hi = sched
    base_lo = np.concatenate([[0], np.cumsum(C_lo)[:-1]]).astype(int)
    base_hi = np.concatenate([[0], np.cumsum(C_hi)[:-1]]).astype(int)
    n_lo, n_hi = int(sum(C_lo)), int(sum(C_hi))
    cons_base = np.concatenate(
        [[0], np.cumsum(np.array(C_lo) + np.array(C_hi))[:-1]]
    ).astype(int)
    ncp = (nch + W - 1) // W * W
    f16 = mybir.dt.float16
    f32 = mybir.dt.float32
    AF = mybir.ActivationFunctionType
    ALU = mybir.AluOpType

    nc = bacc.Bacc("TRN2", target_bir_lowering=False, debug=False,
                   num_devices=N_CORES, num_swdge_queues=4)

    Xa_d = nc.dram_tensor("Xa", [NA, D], f16, kind="ExternalInput")
    Xb_d = nc.dram_tensor("Xb", [NB, D], f16, kind="ExternalInput")
    W1_d = nc.dram_tensor("W1f", [128, 128], f16, kind="ExternalInput")
    W2_d = nc.dram_tensor("W2f", [128, 128], f16, kind="ExternalInput")
    b1_d = nc.dram_tensor("b1r", [1, 128], f16, kind="ExternalInput")
    b2_d = nc.dram_tensor("b2r", [1, 128], f16, kind="ExternalInput")
    on_d = nc.dram_tensor("ones", [1, 128], f16, kind="ExternalInput")
    io_d = nc.dram_tensor("iota", [128, 128], f16, kind="ExternalInput")
    ilo_d = nc.dram_tensor("idx_lo", [128, n_lo * 8], mybir.dt.int16,
                           kind="ExternalInput")
    ihi_d = nc.dram_tensor("idx_hi", [128, n_hi * 8], mybir.dt.int16,
                           kind="ExternalInput")
    mdst_d = nc.dram_tensor("meta_dst", [128, nch], f32, kind="ExternalInput")
    mnrm_d = nc.dram_tensor("meta_nrm", [128, nch], f32, kind="ExternalInput")

    rparts = (
        nc.dram_tensor("r_a", [RS, D], f16, kind="Internal"),
        nc.dram_tensor("r_b", [NPC - RS, D], f16, kind="Internal"),
    )
    Ra = nc.dram_tensor("Ra", [NA, D], f16, kind="Internal",
                        addr_space="Shared")
    Rb = nc.dram_tensor("Rb", [NB, D], f16, kind="Internal",
                        addr_space="Shared")
    out_d = nc.dram_tensor("outN", [NT * 128, D], f32, kind="ExternalOutput")

    qctr = [0]

    def next_q():
        q = qctr[0] % 4
        qctr[0] += 1
        return q

    with tile.TileContext(nc) as tc:
        with (
            tc.tile_pool(name="consts", bufs=1) as consts,
            tc.tile_pool(name="mt", bufs=24) as mt_pool,
            tc.tile_pool(name="st", bufs=96) as st_pool,
            tc.tile_pool(name="hb", bufs=6) as hb_pool,
            tc.tile_pool(name="pf", bufs=4) as pf_pool,
            tc.tile_pool(name="psa", bufs=3, space="PSUM") as psa_pool,
            tc.tile_pool(name="ps", bufs=5, space="PSUM") as ps_pool,
        ):
            nc.gpsimd.load_library(mlp)

            W1f = consts.tile([128, 128], f16, tag="W1f")
            W2f = consts.tile([128, 128], f16, tag="W2f")
            b1r = consts.tile([1, 128], f16, tag="b1r")
            b2r = consts.tile([1, 128], f16, tag="b2r")
            ones = consts.tile([1, 128], f16, tag="ones")
            iota = consts.tile([128, 128], f16, tag="iota")
            idx_lo = consts.tile([128, n_lo * 8], mybir.dt.int16, tag="ilo")
            idx_hi = consts.tile([128, n_hi * 8], mybir.dt.int16, tag="ihi")
            mdst = consts.tile([128, nch], f32, tag="mdst")
            mnrm = consts.tile([128, nch], f32, tag="mnrm")
            nc.sync.dma_start(idx_lo[:], ilo_d.ap())
            nc.sync.dma_start(mdst[:], mdst_d.ap())
            nc.sync.dma_start(mnrm[:], mnrm_d.ap())
            nc.sync.dma_start(idx_hi[:], ihi_d.ap())
            nc.sync.dma_start(W1f[:], W1_d.ap())
            nc.sync.dma_start(W2f[:], W2_d.ap())
            nc.sync.dma_start(b1r[:], b1_d.ap())
            nc.sync.dma_start(b2r[:], b2_d.ap())
            nc.sync.dma_start(ones[:], on_d.ap())
            nc.sync.dma_start(iota[:], io_d.ap())

            BT = 4
            TSPLIT = RS // 128      # 32

            def make_h_writer(h_dram, t_lo, t_hi):
                """Write tiles [t_lo, t_hi) into h_dram (rows rebased)."""
                nfull = min(t_hi, NPC // 128) - t_lo
                h3 = h_dram.ap()[0 : nfull * 128, :].rearrange(
                    "(a p) d -> p a d", p=128
                )
                state = {}

                def write(t, produce):
                    tl_ = t - t_lo
                    if tl_ < nfull:
                        g = tl_ - tl_ % BT
                        if tl_ % BT == 0:
                            state["buf"] = hb_pool.tile(
                                [128, BT, 128], f16, tag="hstage", name="hstage"
                            )
                        produce(state["buf"][:, tl_ % BT, :])
                        if tl_ % BT == BT - 1 or tl_ == nfull - 1:
                            n = tl_ - g + 1
                            nc.scalar.dma_start(h3[:, g : g + n, :],
                                                state["buf"][:, 0:n, :])
                    else:
                        rows = NPC - t * 128
                        tl = hb_pool.tile([128, 128], f16, tag="hrag",
                                          name="hrag")
                        produce(tl[:])
                        nc.scalar.dma_start(
                            h_dram.ap()[tl_ * 128 : tl_ * 128 + rows, :],
                            tl[:rows, :],
                        )

                return write

            def sub_allgather(h_sub, H_out):
                nc.gpsimd.collective_compute(
                    "AllGather", mybir.AluOpType.bypass,
                    replica_groups=[list(range(N_CORES))],
                    ins=[h_sub.ap()], outs=[H_out.ap()],
                )

            def agg_phase(Ta, Tb, out_cb):
                """Gather in fixed W-chunk windows per (lo, hi) stream; per
                dst tile accumulate segment-sum matmuls into psum [f x d],
                then out_cb(t, pa). Selector built on DVE per S-window."""
                streams = {"lo": (Ta, idx_lo, n_lo), "hi": (Tb, idx_hi, n_hi)}
                mt_tiles = {}
                st_tiles = {}

                def ensure_window(s, w):
                    key = (s, w)
                    if key in mt_tiles:
                        return
                    tab, idx_t, n_s = streams[s]
                    cb = w * W
                    cw = min(W, n_s - cb)
                    mt = mt_pool.tile([128, cw, 128], f16, tag="mt")
                    nc.gpsimd.dma_gather(
                        mt[:], tab.ap(),
                        idx_t[:, cb * 8 : (cb + cw) * 8],
                        cw * 128, cw * 128, 128,
                        queue_num=next_q(),
                    )
                    mt_tiles[key] = mt

                def ensure_s(cons):
                    if cons not in st_tiles:
                        st = st_pool.tile([128, 128], f16, tag="st")
                        nc.vector.tensor_scalar(
                            st[:], iota[:], mdst[:, cons : cons + 1],
                            mnrm[:, cons : cons + 1], ALU.is_equal, ALU.mult)
                        st_tiles[cons] = st
                    return st_tiles[cons][:]

                # Prefetch lo windows so the in-order Pool engine has queued
                # DMA work to drain while the first hi window waits on the
                # part-b table.
                for w in range(min(16, (n_lo + W - 1) // W)):
                    ensure_window("lo", w)

                for t in range(NT):
                    nlo, nhi = C_lo[t], C_hi[t]
                    pa = ps_pool.tile([128, 128], f32, tag="pa")
                    for i in range(nlo + nhi):
                        if i < nlo:
                            s, sc = "lo", int(base_lo[t]) + i
                        else:
                            s, sc = "hi", int(base_hi[t]) + (i - nlo)
                        w, o = sc // W, sc % W
                        ensure_window(s, w)
                        cons = int(cons_base[t]) + i
                        s_ap = ensure_s(cons)
                        nc.tensor.matmul(
                            pa[:], mt_tiles[(s, w)][:, o, :], s_ap,
                            start=(i == 0), stop=(i == nlo + nhi - 1),
                        )
                    out_cb(t, pa)

            # ---- layer 1: agg(x) -> @W1 + b1 -> relu -> r parts + AG ----
            w1_a = make_h_writer(rparts[0], 0, TSPLIT)
            w1_b = make_h_writer(rparts[1], TSPLIT, NT)

            def phase_1(t, pa):
                paf = pf_pool.tile([128, 128], f16, tag="paf")
                nc.scalar.activation(paf[:], pa[:], AF.Copy)
                ph = psa_pool.tile([128, 128], f32, tag="ph", name="ph")
                nc.tensor.matmul(ph[:], paf[:], W1f[:], start=True, stop=False)
                nc.tensor.matmul(ph[:], ones[:], b1r[:], start=False, stop=True)
                (w1_a if t < TSPLIT else w1_b)(
                    t, lambda dst, ph=ph: nc.scalar.activation(
                        dst, ph[:], AF.Relu))
                if t == TSPLIT - 1:
                    sub_allgather(rparts[0], Ra)

            agg_phase(Xa_d, Xb_d, phase_1)
            sub_allgather(rparts[1], Rb)

            # ---- layer 2: agg(r) -> @W2 + b2 -> out (node-major) ----
            o3 = out_d.ap().rearrange("(a p) d -> p a d", p=128)
            ostate = {}

            def phase_2(t, pa):
                paf = pf_pool.tile([128, 128], f16, tag="paf")
                nc.scalar.activation(paf[:], pa[:], AF.Copy)
                ph = psa_pool.tile([128, 128], f32, tag="ph", name="ph2")
                nc.tensor.matmul(ph[:], paf[:], W2f[:], start=True, stop=False)
                nc.tensor.matmul(ph[:], ones[:], b2r[:], start=False, stop=True)
                g = t - t % BT
                if t % BT == 0:
                    ostate["buf"] = hb_pool.tile([128, BT, 128], f32,
                                                 tag="ostage", name="ostage")
                nc.scalar.activation(ostate["buf"][:, t % BT, :], ph[:],
                                     AF.Copy)
                if t % BT == BT - 1 or t == NT - 1:
                    n = t - g + 1
                    nc.scalar.dma_start(o3[:, g : g + n, :],
                                        ostate["buf"][:, 0:n, :])

            agg_phase(Ra, Rb, phase_2)

    nc.compile()
    return nc


def kernel(x, src, dst, W1, b1, W2, b2):
    from concourse.bass_utils import run_bass_kernel_spmd

    in_maps, sched, nch = _host_prep(x, src, dst, W1, b1, W2, b2)
    key = (sched, nch)
    if key not in _CACHE:
        _CACHE[key] = _build_program(sched, nch)
    nc = _CACHE[key]
    res = run_bass_kernel_spmd(nc, in_maps, core_ids=list(range(N_CORES)))
    out = np.empty((N_NODES, D), dtype=np.float32)
    for k in range(N_CORES):
        out[k * NPC : (k + 1) * NPC] = res.results[k]["outN"][:NPC]
    return out


# revision 46
# speedup vs baseline: 1.1010x; 1.0449x over previous
"""Two-layer GCN (GraphConv norm='both') on 8 Trainium2 NeuronCores.

Strategy (graph/data parallel per the sharding hint):
  - dst nodes partitioned across 8 cores (6250 rows each); weights replicated.
  - The dense weight is COMMUTED through the segment-sum:
    sum_e ns[s]*nd[d]*(x[s] @ W) = (sum_e ns[s]*nd[d]*x[s]) @ W, so layer 1
    aggregates RAW x rows. Each core receives the full x as a plain input
    split into two HBM tables (no phase-A compute, no layer-1 collective:
    gathers start immediately). Layer 2 aggregates relu rows r = relu(agg1
    @ W1 + b1) and applies W2 + b2 after its segment-sum; only the r table
    needs the halo-exchange AllGather (one per layer boundary).
  - Edge aggregation per core: edges grouped by (dst-tile, table-half) and
    sorted by source; source rows fetched with SWDGE dma_gather (256B f16
    rows, 8-chunk single-packet windows rotating over 4 SWDGE queues);
    segment-sum on the PE as one-hot matmuls into PSUM [f_in x dst]. The
    edge-weight selector S (carrying ns*nd) is built ON-CHIP on the DVE:
    S = (iota == dstcol) * nrm from a tiny per-chunk metadata table.
  - int16 gather indices cover only 32768 rows, so tables are split at row
    32768 (part a = each core's rows 0..4095 concatenated, part b = the
    rest). 256B single-row gathers have zero fetch waste, and part-a
    gathers depend only on the part-a AllGather (Tile tracks DRAM deps per
    tensor), letting next-layer gathers start while part b is in flight.
  - Biases are rank-1 matmul updates (ones-column x bias-row) into the
    same PSUM tile as the dense matmul; per-tile flow is
    pa=[f,d] -> f16 -> @W -> [d,f] node-major (+bias) -> relu/copy.

All feature math runs on-device in fp16 (f32 PSUM accumulation); index
preprocessing (edge partitioning/sorting, degree counts, gather schedule)
is host-side sharding prep.
"""

import numpy as np

N_NODES = 50000
N_EDGES = 600000
D = 128
N_CORES = 8
NPC = N_NODES // N_CORES          # 6250 nodes per core
NT = (NPC + 127) // 128           # 49 dst tiles per core
RS = 4096                         # part-a rows per core (8*RS = 32768)
NA = N_CORES * RS                 # 32768 rows in table a
NB = N_NODES - NA                 # 17232 rows in table b
W = 8                             # gather window size (chunks per call)

_CACHE = {}


def _host_prep(x, src, dst, W1, b1, W2, b2):
    x = np.asarray(x, dtype=np.float32)
    src = np.asarray(src, dtype=np.int64)
    dst = np.asarray(dst, dtype=np.int64)
    W1 = np.asarray(W1, dtype=np.float32)
    W2 = np.asarray(W2, dtype=np.float32)
    b1 = np.asarray(b1, dtype=np.float32)
    b2 = np.asarray(b2, dtype=np.float32)

    deg_out = np.bincount(src, minlength=N_NODES).astype(np.float32)
    deg_in = np.bincount(dst, minlength=N_NODES).astype(np.float32)
    norm_src = np.where(deg_out > 0, 1.0 / np.sqrt(np.maximum(deg_out, 1.0)), 0.0)
    norm_dst = np.where(deg_in > 0, 1.0 / np.sqrt(np.maximum(deg_in, 1.0)), 0.0)
    norm_src = norm_src.astype(np.float32)
    norm_dst = norm_dst.astype(np.float32)

    # Map node -> row in the split-table layout: part a = each core's rows
    # [0, RS) concatenated; part b = rows [RS, NPC) concatenated.
    nodes = np.arange(N_NODES, dtype=np.int64)
    ksn, rsn = nodes // NPC, nodes % NPC
    cat_of_node = np.where(rsn < RS, ksn * RS + rsn,
                           NA + ksn * (NPC - RS) + (rsn - RS))
    xcat = np.empty((N_NODES, D), dtype=np.float16)
    xcat[cat_of_node] = x.astype(np.float16)
    Xa, Xb = np.ascontiguousarray(xcat[:NA]), np.ascontiguousarray(xcat[NA:])

    cat = cat_of_node[src]
    half = (cat >= NA).astype(np.int64)

    # --- per-core edge grouping by (dst tile, half), sorted by cat ---
    per_core = []
    cnts = np.zeros((N_CORES, NT, 2), dtype=np.int64)
    for k in range(N_CORES):
        m = (dst >= k * NPC) & (dst < (k + 1) * NPC)
        c_k = cat[m]
        s_k = src[m]
        dl_k = dst[m] - k * NPC
        t_k = dl_k >> 7
        h_k = half[m]
        key = t_k * 2 + h_k
        order = np.lexsort((c_k, key))
        per_core.append((c_k[order], s_k[order], dl_k[order], key[order],
                         h_k[order]))
        cnts[k] = np.bincount(key, minlength=NT * 2).reshape(NT, 2)

    # shared static schedule: chunks per (tile, half), max over cores
    C_lo = np.maximum.reduce([(cnts[k, :, 0] + 127) // 128 for k in range(N_CORES)])
    C_hi = np.maximum.reduce([(cnts[k, :, 1] + 127) // 128 for k in range(N_CORES)])
    C_lo = np.where((C_lo + C_hi) == 0, 1, C_lo)
    base_lo = np.concatenate([[0], np.cumsum(C_lo)[:-1]])
    base_hi = np.concatenate([[0], np.cumsum(C_hi)[:-1]])
    n_lo, n_hi = int(C_lo.sum()), int(C_hi.sum())
    cons_base = np.concatenate([[0], np.cumsum(C_lo + C_hi)[:-1]])
    nch = n_lo + n_hi

    in_maps = []
    for k in range(N_CORES):
        c_k, s_k, dl_k, key, h_k = per_core[k]
        t_k = key >> 1
        grp_counts = np.bincount(key, minlength=NT * 2)
        grp_start = np.concatenate([[0], np.cumsum(grp_counts)[:-1]])
        rank = np.arange(len(key)) - grp_start[key]
        chunk_in_grp = rank >> 7
        lo_m = h_k == 0
        pos = np.where(lo_m, base_lo[t_k] * 128, base_hi[t_k] * 128) + rank

        idx_lo = np.zeros(n_lo * 128, dtype=np.int16)
        idx_hi = np.zeros(n_hi * 128, dtype=np.int16)
        idx_lo[pos[lo_m]] = c_k[lo_m].astype(np.int16)
        idx_hi[pos[~lo_m]] = (c_k[~lo_m] - NA).astype(np.int16)

        # consumption order per tile: lo chunks then hi chunks
        col = cons_base[t_k] + np.where(lo_m, chunk_in_grp,
                                        C_lo[t_k] + chunk_in_grp)
        row = rank & 127
        ncp = (nch + W - 1) // W * W
        meta_dst = np.full((128, ncp), 999.0, dtype=np.float16)
        meta_nrm = np.zeros((128, ncp), dtype=np.float16)
        meta_dst[row, col] = (dl_k & 127).astype(np.float16)
        meta_nrm[row, col] = (norm_src[s_k]
                              * norm_dst[dl_k + k * NPC]).astype(np.float16)

        in_maps.append(
            {
                "Xa": Xa,
                "Xb": Xb,
                "W1f": W1.astype(np.float16),
                "W2f": W2.astype(np.float16),
                "b1r": b1.reshape(1, 128).astype(np.float16),
                "b2r": b2.reshape(1, 128).astype(np.float16),
                "ones": np.ones((1, 128), dtype=np.float16),
                "iota8": np.tile(np.arange(128, dtype=np.float16), (128, W)),
                "idx_lo": np.tile(idx_lo.reshape(-1, 16).T, (8, 1)),
                "idx_hi": np.tile(idx_hi.reshape(-1, 16).T, (8, 1)),
                "meta_dst": meta_dst,
                "meta_nrm": meta_nrm,
            }
        )
    sched = (tuple(int(v) for v in C_lo), tuple(int(v) for v in C_hi))
    return in_maps, sched, nch


def _build_program(sched, nch):
    import concourse.bacc as bacc
    import concourse.mybir as mybir
    import concourse.tile as tile
    from concourse.library_config import mlp

    C_lo, C_hi = sched
    base_lo = np.concatenate([[0], np.cumsum(C_lo)[:-1]]).astype(int)
    base_hi = np.concatenate([[0], np.cumsum(C_hi)[:-1]]).astype(int)
    n_lo, n_hi = int(sum(C_lo)), int(sum(C_hi))
    cons_base = np.concatenate(
        [[0], np.cumsum(np.array(C_lo) + np.array(C_hi))[:-1]]
    ).astype(int)
    ncp = (nch + W - 1) // W * W
    f16 = mybir.dt.float16
    f32 = mybir.dt.float32
    AF = mybir.ActivationFunctionType
    ALU = mybir.AluOpType

    nc = bacc.Bacc("TRN2", target_bir_lowering=False, debug=False,
                   num_devices=N_CORES, num_swdge_queues=4)

    Xa_d = nc.dram_tensor("Xa", [NA, D], f16, kind="ExternalInput")
    Xb_d = nc.dram_tensor("Xb", [NB, D], f16, kind="ExternalInput")
    W1_d = nc.dram_tensor("W1f", [128, 128], f16, kind="ExternalInput")
    W2_d = nc.dram_tensor("W2f", [128, 128], f16, kind="ExternalInput")
    b1_d = nc.dram_tensor("b1r", [1, 128], f16, kind="ExternalInput")
    b2_d = nc.dram_tensor("b2r", [1, 128], f16, kind="ExternalInput")
    on_d = nc.dram_tensor("ones", [1, 128], f16, kind="ExternalInput")
    io_d = nc.dram_tensor("iota8", [128, W * 128], f16, kind="ExternalInput")
    ilo_d = nc.dram_tensor("idx_lo", [128, n_lo * 8], mybir.dt.int16,
                           kind="ExternalInput")
    ihi_d = nc.dram_tensor("idx_hi", [128, n_hi * 8], mybir.dt.int16,
                           kind="ExternalInput")
    mdst_d = nc.dram_tensor("meta_dst", [128, ncp], f16, kind="ExternalInput")
    mnrm_d = nc.dram_tensor("meta_nrm", [128, ncp], f16, kind="ExternalInput")

    rparts = (
        nc.dram_tensor("r_a", [RS, D], f16, kind="Internal"),
        nc.dram_tensor("r_b", [NPC - RS, D], f16, kind="Internal"),
    )
    Ra = nc.dram_tensor("Ra", [NA, D], f16, kind="Internal",
                        addr_space="Shared")
    Rb = nc.dram_tensor("Rb", [NB, D], f16, kind="Internal",
                        addr_space="Shared")
    out_d = nc.dram_tensor("outN", [NT * 128, D], f32, kind="ExternalOutput")

    qctr = [0]

    def next_q():
        q = qctr[0] % 4
        qctr[0] += 1
        return q

    with tile.TileContext(nc) as tc:
        with (
            tc.tile_pool(name="consts", bufs=1) as consts,
            tc.tile_pool(name="mt", bufs=24) as mt_pool,
            tc.tile_pool(name="st", bufs=12) as st_pool,
            tc.tile_pool(name="hb", bufs=6) as hb_pool,
            tc.tile_pool(name="pf", bufs=4) as pf_pool,
            tc.tile_pool(name="psa", bufs=3, space="PSUM") as psa_pool,
            tc.tile_pool(name="ps", bufs=5, space="PSUM") as ps_pool,
        ):
            nc.gpsimd.load_library(mlp)

            W1f = consts.tile([128, 128], f16, tag="W1f")
            W2f = consts.tile([128, 128], f16, tag="W2f")
            b1r = consts.tile([1, 128], f16, tag="b1r")
            b2r = consts.tile([1, 128], f16, tag="b2r")
            ones = consts.tile([1, 128], f16, tag="ones")
            iota8 = consts.tile([128, W, 128], f16, tag="iota8")
            idx_lo = consts.tile([128, n_lo * 8], mybir.dt.int16, tag="ilo")
            idx_hi = consts.tile([128, n_hi * 8], mybir.dt.int16, tag="ihi")
            mdst = consts.tile([128, ncp], f16, tag="mdst")
            mnrm = consts.tile([128, ncp], f16, tag="mnrm")
            nc.sync.dma_start(idx_lo[:], ilo_d.ap())
            nc.sync.dma_start(mdst[:], mdst_d.ap())
            nc.sync.dma_start(mnrm[:], mnrm_d.ap())
            nc.sync.dma_start(idx_hi[:], ihi_d.ap())
            nc.sync.dma_start(W1f[:], W1_d.ap())
            nc.sync.dma_start(W2f[:], W2_d.ap())
            nc.sync.dma_start(b1r[:], b1_d.ap())
            nc.sync.dma_start(b2r[:], b2_d.ap())
            nc.sync.dma_start(ones[:], on_d.ap())
            nc.sync.dma_start(iota8[:],
                              io_d.ap().rearrange("p (a e) -> p a e", a=W))

            BT = 4
            TSPLIT = RS // 128      # 32

            def make_h_writer(h_dram, t_lo, t_hi):
                """Write tiles [t_lo, t_hi) into h_dram (rows rebased)."""
                nfull = min(t_hi, NPC // 128) - t_lo
                h3 = h_dram.ap()[0 : nfull * 128, :].rearrange(
                    "(a p) d -> p a d", p=128
                )
                state = {}

                def write(t, produce):
                    tl_ = t - t_lo
                    if tl_ < nfull:
                        g = tl_ - tl_ % BT
                        if tl_ % BT == 0:
                            state["buf"] = hb_pool.tile(
                                [128, BT, 128], f16, tag="hstage", name="hstage"
                            )
                        produce(state["buf"][:, tl_ % BT, :])
                        if tl_ % BT == BT - 1 or tl_ == nfull - 1:
                            n = tl_ - g + 1
                            nc.scalar.dma_start(h3[:, g : g + n, :],
                                                state["buf"][:, 0:n, :])
                    else:
                        rows = NPC - t * 128
                        tl = hb_pool.tile([128, 128], f16, tag="hrag",
                                          name="hrag")
                        produce(tl[:])
                        nc.scalar.dma_start(
                            h_dram.ap()[tl_ * 128 : tl_ * 128 + rows, :],
                            tl[:rows, :],
                        )

                return write

            def sub_allgather(h_sub, H_out):
                nc.gpsimd.collective_compute(
                    "AllGather", mybir.AluOpType.bypass,
                    replica_groups=[list(range(N_CORES))],
                    ins=[h_sub.ap()], outs=[H_out.ap()],
                )

            def agg_phase(Ta, Tb, out_cb):
                """Gather in fixed W-chunk windows per (lo, hi) stream; per
                dst tile accumulate segment-sum matmuls into psum [f x d],
                then out_cb(t, pa). Selector built on DVE per S-window."""
                streams = {"lo": (Ta, idx_lo, n_lo), "hi": (Tb, idx_hi, n_hi)}
                mt_tiles = {}
                st_tiles = {}

                def ensure_window(s, w):
                    key = (s, w)
                    if key in mt_tiles:
                        return
                    tab, idx_t, n_s = streams[s]
                    cb = w * W
                    cw = min(W, n_s - cb)
                    mt = mt_pool.tile([128, cw, 128], f16, tag="mt")
                    nc.gpsimd.dma_gather(
                        mt[:], tab.ap(),
                        idx_t[:, cb * 8 : (cb + cw) * 8],
                        cw * 128, cw * 128, 128,
                        queue_num=next_q(),
                    )
                    mt_tiles[key] = mt

                def ensure_s(cons):
                    sw, so = cons // W, cons % W
                    if sw not in st_tiles:
                        st = st_pool.tile([128, W, 128], f16, tag="st")
                        md = mdst[:, sw * W : (sw + 1) * W].rearrange(
                            "p (a b) -> p a b", b=1).to_broadcast([128, W, 128])
                        mn = mnrm[:, sw * W : (sw + 1) * W].rearrange(
                            "p (a b) -> p a b", b=1).to_broadcast([128, W, 128])
                        nc.vector.tensor_tensor(out=st[:], in0=iota8[:],
                                                in1=md, op=ALU.is_equal)
                        nc.vector.tensor_tensor(out=st[:], in0=st[:],
                                                in1=mn, op=ALU.mult)
                        st_tiles[sw] = st
                    return st_tiles[sw][:, so, :]

                # Prefetch lo windows so the in-order Pool engine has queued
                # DMA work to drain while the first hi window waits on the
                # part-b table.
                for w in range(min(16, (n_lo + W - 1) // W)):
                    ensure_window("lo", w)

                for t in range(NT):
                    nlo, nhi = C_lo[t], C_hi[t]
                    pa = ps_pool.tile([128, 128], f32, tag="pa")
                    for i in range(nlo + nhi):
                        if i < nlo:
                            s, sc = "lo", int(base_lo[t]) + i
                        else:
                            s, sc = "hi", int(base_hi[t]) + (i - nlo)
                        w, o = sc // W, sc % W
                        ensure_window(s, w)
                        cons = int(cons_base[t]) + i
                        s_ap = ensure_s(cons)
                        nc.tensor.matmul(
                            pa[:], mt_tiles[(s, w)][:, o, :], s_ap,
                            start=(i == 0), stop=(i == nlo + nhi - 1),
                        )
                    out_cb(t, pa)

            # ---- layer 1: agg(x) -> @W1 + b1 -> relu -> r parts + AG ----
            w1_a = make_h_writer(rparts[0], 0, TSPLIT)
            w1_b = make_h_writer(rparts[1], TSPLIT, NT)

            def phase_1(t, pa):
                paf = pf_pool.tile([128, 128], f16, tag="paf")
                nc.scalar.activation(paf[:], pa[:], AF.Copy)
                ph = psa_pool.tile([128, 128], f32, tag="ph", name="ph")
                nc.tensor.matmul(ph[:], paf[:], W1f[:], start=True, stop=False)
                nc.tensor.matmul(ph[:], ones[:], b1r[:], start=False, stop=True)
                (w1_a if t < TSPLIT else w1_b)(
                    t, lambda dst, ph=ph: nc.scalar.activation(
                        dst, ph[:], AF.Relu))
                if t == TSPLIT - 1:
                    sub_allgather(rparts[0], Ra)

            agg_phase(Xa_d, Xb_d, phase_1)
            sub_allgather(rparts[1], Rb)

            # ---- layer 2: agg(r) -> @W2 + b2 -> out (node-major) ----
            o3 = out_d.ap().rearrange("(a p) d -> p a d", p=128)
            ostate = {}

            def phase_2(t, pa):
                paf = pf_pool.tile([128, 128], f16, tag="paf")
                nc.scalar.activation(paf[:], pa[:], AF.Copy)
                ph = psa_pool.tile([128, 128], f32, tag="ph", name="ph2")
                nc.tensor.matmul(ph[:], paf[:], W2f[:], start=True, stop=False)
                nc.tensor.matmul(ph[:], ones[:], b2r[:], start=False, stop=True)
                g = t - t % BT
                if t % BT == 0:
                    ostate["buf"] = hb_pool.tile([128, BT, 128], f32,
                                                 tag="ostage", name="ostage")
                nc.scalar.activation(ostate["buf"][:, t % BT, :], ph[:],
                                     AF.Copy)
                if t % BT == BT - 1 or t == NT - 1:
                    n = t - g + 1
                    nc.scalar.dma_start(o3[:, g : g + n, :],
                                        ostate["buf"][:, 0:n, :])

            agg_phase(Ra, Rb, phase_2)

    nc.compile()
    return nc


def kernel(x, src, dst, W1, b1, W2, b2):
    from concourse.bass_utils import run_bass_kernel_spmd

    in_maps, sched, nch = _host_prep(x, src, dst, W1, b1, W2, b2)
    key = (sched, nch)
    if key not in _CACHE:
        _CACHE[key] = _build_program(sched, nch)
    nc = _CACHE[key]
    res = run_bass_kernel_spmd(nc, in_maps, core_ids=list(range(N_CORES)))
    out = np.empty((N_NODES, D), dtype=np.float32)
    for k in range(N_CORES):
        out[k * NPC : (k + 1) * NPC] = res.results[k]["outN"][:NPC]
    return out


# revision 48
# speedup vs baseline: 1.1045x; 1.0032x over previous
"""Two-layer GCN (GraphConv norm='both') on 8 Trainium2 NeuronCores.

Strategy (graph/data parallel per the sharding hint):
  - dst nodes partitioned across 8 cores (6250 rows each); weights replicated.
  - The dense weight is COMMUTED through the segment-sum:
    sum_e ns[s]*nd[d]*(x[s] @ W) = (sum_e ns[s]*nd[d]*x[s]) @ W, so layer 1
    aggregates RAW x rows. Each core receives the full x as a plain input
    split into two HBM tables (no phase-A compute, no layer-1 collective:
    gathers start immediately). Layer 2 aggregates relu rows r = relu(agg1
    @ W1 + b1) and applies W2 + b2 after its segment-sum; only the r table
    needs the halo-exchange AllGather (one per layer boundary).
  - Edge aggregation per core: edges grouped by (dst-tile, table-half) and
    sorted by source; source rows fetched with SWDGE dma_gather (256B f16
    rows, 8-chunk single-packet windows rotating over 4 SWDGE queues);
    segment-sum on the PE as one-hot matmuls into PSUM [f_in x dst]. The
    edge-weight selector S (carrying ns*nd) is built ON-CHIP on the DVE:
    S = (iota == dstcol) * nrm from a tiny per-chunk metadata table.
  - int16 gather indices cover only 32768 rows, so tables are split at row
    32768 (part a = each core's rows 0..4095 concatenated, part b = the
    rest). 256B single-row gathers have zero fetch waste, and part-a
    gathers depend only on the part-a AllGather (Tile tracks DRAM deps per
    tensor), letting next-layer gathers start while part b is in flight.
  - Biases are rank-1 matmul updates (ones-column x bias-row) into the
    same PSUM tile as the dense matmul; per-tile flow is
    pa=[f,d] -> f16 -> @W -> [d,f] node-major (+bias) -> relu/copy.

All feature math runs on-device in fp16 (f32 PSUM accumulation); index
preprocessing (edge partitioning/sorting, degree counts, gather schedule)
is host-side sharding prep.
"""

import numpy as np

N_NODES = 50000
N_EDGES = 600000
D = 128
N_CORES = 8
NPC = N_NODES // N_CORES          # 6250 nodes per core
NT = (NPC + 127) // 128           # 49 dst tiles per core
RS = 4096                         # part-a rows per core (8*RS = 32768)
NA = N_CORES * RS                 # 32768 rows in table a
NB = N_NODES - NA                 # 17232 rows in table b
W = 8                             # gather window size (chunks per call)

_CACHE = {}


def _host_prep(x, src, dst, W1, b1, W2, b2):
    x = np.asarray(x, dtype=np.float32)
    src = np.asarray(src, dtype=np.int64)
    dst = np.asarray(dst, dtype=np.int64)
    W1 = np.asarray(W1, dtype=np.float32)
    W2 = np.asarray(W2, dtype=np.float32)
    b1 = np.asarray(b1, dtype=np.float32)
    b2 = np.asarray(b2, dtype=np.float32)

    deg_out = np.bincount(src, minlength=N_NODES).astype(np.float32)
    deg_in = np.bincount(dst, minlength=N_NODES).astype(np.float32)
    norm_src = np.where(deg_out > 0, 1.0 / np.sqrt(np.maximum(deg_out, 1.0)), 0.0)
    norm_dst = np.where(deg_in > 0, 1.0 / np.sqrt(np.maximum(deg_in, 1.0)), 0.0)
    norm_src = norm_src.astype(np.float32)
    norm_dst = norm_dst.astype(np.float32)

    # Map node -> row in the split-table layout: part a = each core's rows
    # [0, RS) concatenated; part b = rows [RS, NPC) concatenated.
    nodes = np.arange(N_NODES, dtype=np.int64)
    ksn, rsn = nodes // NPC, nodes % NPC
    cat_of_node = np.where(rsn < RS, ksn * RS + rsn,
                           NA + ksn * (NPC - RS) + (rsn - RS))
    xcat = np.empty((N_NODES, D), dtype=np.float16)
    xcat[cat_of_node] = x.astype(np.float16)
    Xa, Xb = np.ascontiguousarray(xcat[:NA]), np.ascontiguousarray(xcat[NA:])

    cat = cat_of_node[src]
    half = (cat >= NA).astype(np.int64)

    # --- per-core edge grouping by (dst tile, half), sorted by cat ---
    per_core = []
    cnts = np.zeros((N_CORES, NT, 2), dtype=np.int64)
    for k in range(N_CORES):
        m = (dst >= k * NPC) & (dst < (k + 1) * NPC)
        c_k = cat[m]
        s_k = src[m]
        dl_k = dst[m] - k * NPC
        t_k = dl_k >> 7
        h_k = half[m]
        key = t_k * 2 + h_k
        order = np.lexsort((c_k, key))
        per_core.append((c_k[order], s_k[order], dl_k[order], key[order],
                         h_k[order]))
        cnts[k] = np.bincount(key, minlength=NT * 2).reshape(NT, 2)

    # shared static schedule: chunks per (tile, half), max over cores
    C_lo = np.maximum.reduce([(cnts[k, :, 0] + 127) // 128 for k in range(N_CORES)])
    C_hi = np.maximum.reduce([(cnts[k, :, 1] + 127) // 128 for k in range(N_CORES)])
    C_lo = np.where((C_lo + C_hi) == 0, 1, C_lo)
    base_lo = np.concatenate([[0], np.cumsum(C_lo)[:-1]])
    base_hi = np.concatenate([[0], np.cumsum(C_hi)[:-1]])
    n_lo, n_hi = int(C_lo.sum()), int(C_hi.sum())
    cons_base = np.concatenate([[0], np.cumsum(C_lo + C_hi)[:-1]])
    nch = n_lo + n_hi

    in_maps = []
    for k in range(N_CORES):
        c_k, s_k, dl_k, key, h_k = per_core[k]
        t_k = key >> 1
        grp_counts = np.bincount(key, minlength=NT * 2)
        grp_start = np.concatenate([[0], np.cumsum(grp_counts)[:-1]])
        rank = np.arange(len(key)) - grp_start[key]
        chunk_in_grp = rank >> 7
        lo_m = h_k == 0
        pos = np.where(lo_m, base_lo[t_k] * 128, base_hi[t_k] * 128) + rank

        idx_lo = np.zeros(n_lo * 128, dtype=np.int16)
        idx_hi = np.zeros(n_hi * 128, dtype=np.int16)
        idx_lo[pos[lo_m]] = c_k[lo_m].astype(np.int16)
        idx_hi[pos[~lo_m]] = (c_k[~lo_m] - NA).astype(np.int16)

        # consumption order per tile: lo chunks then hi chunks
        col = cons_base[t_k] + np.where(lo_m, chunk_in_grp,
                                        C_lo[t_k] + chunk_in_grp)
        row = rank & 127
        ncp = (nch + W - 1) // W * W
        meta_dst = np.full((128, ncp), 999.0, dtype=np.float16)
        meta_nrm = np.zeros((128, ncp), dtype=np.float16)
        meta_dst[row, col] = (dl_k & 127).astype(np.float16)
        meta_nrm[row, col] = (norm_src[s_k]
                              * norm_dst[dl_k + k * NPC]).astype(np.float16)

        in_maps.append(
            {
                "Xa": Xa,
                "Xb": Xb,
                "W1f": W1.astype(np.float16),
                "W2f": W2.astype(np.float16),
                "b1r": b1.reshape(1, 128).astype(np.float16),
                "b2r": b2.reshape(1, 128).astype(np.float16),
                "ones": np.ones((1, 128), dtype=np.float16),
                "iota8": np.tile(np.arange(128, dtype=np.float16), (128, W)),
                "idx_lo": np.tile(idx_lo.reshape(-1, 16).T, (8, 1)),
                "idx_hi": np.tile(idx_hi.reshape(-1, 16).T, (8, 1)),
                "meta_dst": meta_dst,
                "meta_nrm": meta_nrm,
            }
        )
    sched = (tuple(int(v) for v in C_lo), tuple(int(v) for v in C_hi))
    return in_maps, sched, nch


def _build_program(sched, nch):
    import concourse.bacc as bacc
    import concourse.mybir as mybir
    import concourse.tile as tile
    from concourse.library_config import mlp

    C_lo, C_hi = sched
    base_lo = np.concatenate([[0], np.cumsum(C_lo)[:-1]]).astype(int)
    base_hi = np.concatenate([[0], np.cumsum(C_hi)[:-1]]).astype(int)
    n_lo, n_hi = int(sum(C_lo)), int(sum(C_hi))
    cons_base = np.concatenate(
        [[0], np.cumsum(np.array(C_lo) + np.array(C_hi))[:-1]]
    ).astype(int)
    ncp = (nch + W - 1) // W * W
    f16 = mybir.dt.float16
    f32 = mybir.dt.float32
    AF = mybir.ActivationFunctionType
    ALU = mybir.AluOpType

    nc = bacc.Bacc("TRN2", target_bir_lowering=False, debug=False,
                   num_devices=N_CORES, num_swdge_queues=4)

    Xa_d = nc.dram_tensor("Xa", [NA, D], f16, kind="ExternalInput")
    Xb_d = nc.dram_tensor("Xb", [NB, D], f16, kind="ExternalInput")
    W1_d = nc.dram_tensor("W1f", [128, 128], f16, kind="ExternalInput")
    W2_d = nc.dram_tensor("W2f", [128, 128], f16, kind="ExternalInput")
    b1_d = nc.dram_tensor("b1r", [1, 128], f16, kind="ExternalInput")
    b2_d = nc.dram_tensor("b2r", [1, 128], f16, kind="ExternalInput")
    on_d = nc.dram_tensor("ones", [1, 128], f16, kind="ExternalInput")
    io_d = nc.dram_tensor("iota8", [128, W * 128], f16, kind="ExternalInput")
    ilo_d = nc.dram_tensor("idx_lo", [128, n_lo * 8], mybir.dt.int16,
                           kind="ExternalInput")
    ihi_d = nc.dram_tensor("idx_hi", [128, n_hi * 8], mybir.dt.int16,
                           kind="ExternalInput")
    mdst_d = nc.dram_tensor("meta_dst", [128, ncp], f16, kind="ExternalInput")
    mnrm_d = nc.dram_tensor("meta_nrm", [128, ncp], f16, kind="ExternalInput")

    rparts = (
        nc.dram_tensor("r_a", [RS, D], f16, kind="Internal"),
        nc.dram_tensor("r_b", [NPC - RS, D], f16, kind="Internal"),
    )
    Ra = nc.dram_tensor("Ra", [NA, D], f16, kind="Internal",
                        addr_space="Shared")
    Rb = nc.dram_tensor("Rb", [NB, D], f16, kind="Internal",
                        addr_space="Shared")
    out_d = nc.dram_tensor("outN", [NT * 128, D], f32, kind="ExternalOutput")

    qctr = [0]

    def next_q():
        q = qctr[0] % 4
        qctr[0] += 1
        return q

    with tile.TileContext(nc) as tc:
        with (
            tc.tile_pool(name="consts", bufs=1) as consts,
            tc.tile_pool(name="mt", bufs=24) as mt_pool,
            tc.tile_pool(name="st", bufs=16) as st_pool,
            tc.tile_pool(name="hb", bufs=6) as hb_pool,
            tc.tile_pool(name="pf", bufs=4) as pf_pool,
            tc.tile_pool(name="psa", bufs=3, space="PSUM") as psa_pool,
            tc.tile_pool(name="ps", bufs=5, space="PSUM") as ps_pool,
        ):
            nc.gpsimd.load_library(mlp)

            W1f = consts.tile([128, 128], f16, tag="W1f")
            W2f = consts.tile([128, 128], f16, tag="W2f")
            b1r = consts.tile([1, 128], f16, tag="b1r")
            b2r = consts.tile([1, 128], f16, tag="b2r")
            ones = consts.tile([1, 128], f16, tag="ones")
            iota8 = consts.tile([128, W, 128], f16, tag="iota8")
            idx_lo = consts.tile([128, n_lo * 8], mybir.dt.int16, tag="ilo")
            idx_hi = consts.tile([128, n_hi * 8], mybir.dt.int16, tag="ihi")
            mdst = consts.tile([128, ncp], f16, tag="mdst")
            mnrm = consts.tile([128, ncp], f16, tag="mnrm")
            nc.sync.dma_start(idx_lo[:], ilo_d.ap())
            nc.sync.dma_start(mdst[:], mdst_d.ap())
            nc.sync.dma_start(mnrm[:], mnrm_d.ap())
            nc.sync.dma_start(idx_hi[:], ihi_d.ap())
            nc.sync.dma_start(W1f[:], W1_d.ap())
            nc.sync.dma_start(W2f[:], W2_d.ap())
            nc.sync.dma_start(b1r[:], b1_d.ap())
            nc.sync.dma_start(b2r[:], b2_d.ap())
            nc.sync.dma_start(ones[:], on_d.ap())
            nc.sync.dma_start(iota8[:],
                              io_d.ap().rearrange("p (a e) -> p a e", a=W))

            BT = 4
            TSPLIT = RS // 128      # 32

            def make_h_writer(h_dram, t_lo, t_hi):
                """Write tiles [t_lo, t_hi) into h_dram (rows rebased)."""
                nfull = min(t_hi, NPC // 128) - t_lo
                h3 = h_dram.ap()[0 : nfull * 128, :].rearrange(
                    "(a p) d -> p a d", p=128
                )
                state = {}

                def write(t, produce):
                    tl_ = t - t_lo
                    if tl_ < nfull:
                        g = tl_ - tl_ % BT
                        if tl_ % BT == 0:
                            state["buf"] = hb_pool.tile(
                                [128, BT, 128], f16, tag="hstage", name="hstage"
                            )
                        produce(state["buf"][:, tl_ % BT, :])
                        if tl_ % BT == BT - 1 or tl_ == nfull - 1:
                            n = tl_ - g + 1
                            nc.scalar.dma_start(h3[:, g : g + n, :],
                                                state["buf"][:, 0:n, :])
                    else:
                        rows = NPC - t * 128
                        tl = hb_pool.tile([128, 128], f16, tag="hrag",
                                          name="hrag")
                        produce(tl[:])
                        nc.scalar.dma_start(
                            h_dram.ap()[tl_ * 128 : tl_ * 128 + rows, :],
                            tl[:rows, :],
                        )

                return write

            def sub_allgather(h_sub, H_out):
                nc.gpsimd.collective_compute(
                    "AllGather", mybir.AluOpType.bypass,
                    replica_groups=[list(range(N_CORES))],
                    ins=[h_sub.ap()], outs=[H_out.ap()],
                )

            def agg_phase(Ta, Tb, out_cb):
                """Gather in fixed W-chunk windows per (lo, hi) stream; per
                dst tile accumulate segment-sum matmuls into psum [f x d],
                then out_cb(t, pa). Selector built on DVE per S-window."""
                streams = {"lo": (Ta, idx_lo, n_lo), "hi": (Tb, idx_hi, n_hi)}
                mt_tiles = {}
                st_tiles = {}

                def ensure_window(s, w):
                    key = (s, w)
                    if key in mt_tiles:
                        return
                    tab, idx_t, n_s = streams[s]
                    cb = w * W
                    cw = min(W, n_s - cb)
                    mt = mt_pool.tile([128, cw, 128], f16, tag="mt")
                    nc.gpsimd.dma_gather(
                        mt[:], tab.ap(),
                        idx_t[:, cb * 8 : (cb + cw) * 8],
                        cw * 128, cw * 128, 128,
                        queue_num=next_q(),
                    )
                    mt_tiles[key] = mt

                def ensure_s(cons):
                    sw, so = cons // W, cons % W
                    if sw not in st_tiles:
                        st = st_pool.tile([128, W, 128], f16, tag="st")
                        md = mdst[:, sw * W : (sw + 1) * W].rearrange(
                            "p (a b) -> p a b", b=1).to_broadcast([128, W, 128])
                        mn = mnrm[:, sw * W : (sw + 1) * W].rearrange(
                            "p (a b) -> p a b", b=1).to_broadcast([128, W, 128])
                        nc.vector.tensor_tensor(out=st[:], in0=iota8[:],
                                                in1=md, op=ALU.is_equal)
                        nc.vector.tensor_tensor(out=st[:], in0=st[:],
                                                in1=mn, op=ALU.mult)
                        st_tiles[sw] = st
                    return st_tiles[sw][:, so, :]

                # Prefetch lo windows so the in-order Pool engine has queued
                # DMA work to drain while the first hi window waits on the
                # part-b table.
                for w in range(min(20, (n_lo + W - 1) // W)):
                    ensure_window("lo", w)

                def chunk_list(t):
                    out = []
                    for i in range(C_lo[t] + C_hi[t]):
                        if i < C_lo[t]:
                            out.append(("lo", int(base_lo[t]) + i))
                        else:
                            out.append(("hi", int(base_hi[t]) + (i - C_lo[t])))
                    return out

                for t in range(NT):
                    # Emit tile t+2's gather windows and S-builds ahead so
                    # that by the time their matmuls issue, every semaphore
                    # is already satisfied (fast-path dispatch on the PE).
                    if t + 2 < NT:
                        for i2, (s2, sc2) in enumerate(chunk_list(t + 2)):
                            ensure_window(s2, sc2 // W)
                            ensure_s(int(cons_base[t + 2]) + i2)
                    chunks = chunk_list(t)
                    pa = ps_pool.tile([128, 128], f32, tag="pa")
                    for i, (s, sc) in enumerate(chunks):
                        w, o = sc // W, sc % W
                        ensure_window(s, w)
                        cons = int(cons_base[t]) + i
                        s_ap = ensure_s(cons)
                        nc.tensor.matmul(
                            pa[:], mt_tiles[(s, w)][:, o, :], s_ap,
                            start=(i == 0), stop=(i == len(chunks) - 1),
                        )
                    out_cb(t, pa)

            # ---- layer 1: agg(x) -> @W1 + b1 -> relu -> r parts + AG ----
            w1_a = make_h_writer(rparts[0], 0, TSPLIT)
            w1_b = make_h_writer(rparts[1], TSPLIT, NT)

            def phase_1(t, pa):
                paf = pf_pool.tile([128, 128], f16, tag="paf")
                nc.scalar.activation(paf[:], pa[:], AF.Copy)
                ph = psa_pool.tile([128, 128], f32, tag="ph", name="ph")
                nc.tensor.matmul(ph[:], paf[:], W1f[:], start=True, stop=False)
                nc.tensor.matmul(ph[:], ones[:], b1r[:], start=False, stop=True)
                (w1_a if t < TSPLIT else w1_b)(
                    t, lambda dst, ph=ph: nc.scalar.activation(
                        dst, ph[:], AF.Relu))
                if t == TSPLIT - 1:
                    sub_allgather(rparts[0], Ra)

            agg_phase(Xa_d, Xb_d, phase_1)
            sub_allgather(rparts[1], Rb)

            # ---- layer 2: agg(r) -> @W2 + b2 -> out (node-major) ----
            o3 = out_d.ap().rearrange("(a p) d -> p a d", p=128)
            ostate = {}

            def phase_2(t, pa):
                paf = pf_pool.tile([128, 128], f16, tag="paf")
                nc.scalar.activation(paf[:], pa[:], AF.Copy)
                ph = psa_pool.tile([128, 128], f32, tag="ph", name="ph2")
                nc.tensor.matmul(ph[:], paf[:], W2f[:], start=True, stop=False)
                nc.tensor.matmul(ph[:], ones[:], b2r[:], start=False, stop=True)
                g = t - t % BT
                if t % BT == 0:
                    ostate["buf"] = hb_pool.tile([128, BT, 128], f32,
                                                 tag="ostage", name="ostage")
                nc.scalar.activation(ostate["buf"][:, t % BT, :], ph[:],
                                     AF.Copy)
                if t % BT == BT - 1 or t == NT - 1:
                    n = t - g + 1
                    nc.scalar.dma_start(o3[:, g : g + n, :],
                                        ostate["buf"][:, 0:n, :])

            agg_phase(Ra, Rb, phase_2)

    nc.compile()
    return nc


def kernel(x, src, dst, W1, b1, W2, b2):
    from concourse.bass_utils import run_bass_kernel_spmd

    in_maps, sched, nch = _host_prep(x, src, dst, W1, b1, W2, b2)
    key = (sched, nch)
    if key not in _CACHE:
        _CACHE[key] = _build_program(sched, nch)
    nc = _CACHE[key]
    res = run_bass_kernel_spmd(nc, in_maps, core_ids=list(range(N_CORES)))
    out = np.empty((N_NODES, D), dtype=np.float32)
    for k in range(N_CORES):
        out[k * NPC : (k + 1) * NPC] = res.results[k]["outN"][:NPC]
    return out


# revision 52
# speedup vs baseline: 1.1238x; 1.0175x over previous
"""Two-layer GCN (GraphConv norm='both') on 8 Trainium2 NeuronCores.

Strategy (graph/data parallel per the sharding hint):
  - dst nodes partitioned across 8 cores (6250 rows each); weights replicated.
  - The dense weight is COMMUTED through the segment-sum:
    sum_e ns[s]*nd[d]*(x[s] @ W) = (sum_e ns[s]*nd[d]*x[s]) @ W, so layer 1
    aggregates RAW x rows. Each core receives the full x as a plain input
    split into two HBM tables (no phase-A compute, no layer-1 collective:
    gathers start immediately). Layer 2 aggregates relu rows r = relu(agg1
    @ W1 + b1) and applies W2 + b2 after its segment-sum; only the r table
    needs the halo-exchange AllGather (one per layer boundary).
  - Edge aggregation per core: edges grouped by (dst-tile, table-half) and
    sorted by source; source rows fetched with SWDGE dma_gather (256B f16
    rows, 8-chunk single-packet windows rotating over 4 SWDGE queues);
    segment-sum on the PE as one-hot matmuls into PSUM [f_in x dst]. The
    edge-weight selector S (carrying ns*nd) is built ON-CHIP on the DVE:
    S = (iota == dstcol) * nrm from a tiny per-chunk metadata table.
  - int16 gather indices cover only 32768 rows, so tables are split at row
    32768 (part a = each core's rows 0..4095 concatenated, part b = the
    rest). 256B single-row gathers have zero fetch waste, and part-a
    gathers depend only on the part-a AllGather (Tile tracks DRAM deps per
    tensor), letting next-layer gathers start while part b is in flight.
  - Biases are rank-1 matmul updates (ones-column x bias-row) into the
    same PSUM tile as the dense matmul; per-tile flow is
    pa=[f,d] -> f16 -> @W -> [d,f] node-major (+bias) -> relu/copy.

All feature math runs on-device in fp16 (f32 PSUM accumulation); index
preprocessing (edge partitioning/sorting, degree counts, gather schedule)
is host-side sharding prep.
"""

import numpy as np

N_NODES = 50000
N_EDGES = 600000
D = 128
N_CORES = 8
NPC = N_NODES // N_CORES          # 6250 nodes per core
NT = (NPC + 127) // 128           # 49 dst tiles per core
RS = 4096                         # part-a rows per core (8*RS = 32768)
NA = N_CORES * RS                 # 32768 rows in table a
NB = N_NODES - NA                 # 17232 rows in table b
W = 8                             # gather window size (chunks per call)

_CACHE = {}


def _host_prep(x, src, dst, W1, b1, W2, b2):
    x = np.asarray(x, dtype=np.float32)
    src = np.asarray(src, dtype=np.int64)
    dst = np.asarray(dst, dtype=np.int64)
    W1 = np.asarray(W1, dtype=np.float32)
    W2 = np.asarray(W2, dtype=np.float32)
    b1 = np.asarray(b1, dtype=np.float32)
    b2 = np.asarray(b2, dtype=np.float32)

    deg_out = np.bincount(src, minlength=N_NODES).astype(np.float32)
    deg_in = np.bincount(dst, minlength=N_NODES).astype(np.float32)
    norm_src = np.where(deg_out > 0, 1.0 / np.sqrt(np.maximum(deg_out, 1.0)), 0.0)
    norm_dst = np.where(deg_in > 0, 1.0 / np.sqrt(np.maximum(deg_in, 1.0)), 0.0)
    norm_src = norm_src.astype(np.float32)
    norm_dst = norm_dst.astype(np.float32)

    # Map node -> row in the split-table layout: part a = each core's rows
    # [0, RS) concatenated; part b = rows [RS, NPC) concatenated.
    nodes = np.arange(N_NODES, dtype=np.int64)
    ksn, rsn = nodes // NPC, nodes % NPC
    cat_of_node = np.where(rsn < RS, ksn * RS + rsn,
                           NA + ksn * (NPC - RS) + (rsn - RS))
    xcat = np.empty((N_NODES, D), dtype=np.float16)
    xcat[cat_of_node] = x.astype(np.float16)
    Xa, Xb = np.ascontiguousarray(xcat[:NA]), np.ascontiguousarray(xcat[NA:])

    cat = cat_of_node[src]
    half = (cat >= NA).astype(np.int64)

    # --- per-core edge grouping by (dst tile, half), sorted by cat ---
    per_core = []
    cnts = np.zeros((N_CORES, NT, 2), dtype=np.int64)
    for k in range(N_CORES):
        m = (dst >= k * NPC) & (dst < (k + 1) * NPC)
        c_k = cat[m]
        s_k = src[m]
        dl_k = dst[m] - k * NPC
        t_k = dl_k >> 7
        h_k = half[m]
        key = t_k * 2 + h_k
        order = np.lexsort((c_k, key))
        per_core.append((c_k[order], s_k[order], dl_k[order], key[order],
                         h_k[order]))
        cnts[k] = np.bincount(key, minlength=NT * 2).reshape(NT, 2)

    # shared static schedule: chunks per (tile, half), max over cores
    C_lo = np.maximum.reduce([(cnts[k, :, 0] + 127) // 128 for k in range(N_CORES)])
    C_hi = np.maximum.reduce([(cnts[k, :, 1] + 127) // 128 for k in range(N_CORES)])
    C_lo = np.where((C_lo + C_hi) == 0, 1, C_lo)
    base_lo = np.concatenate([[0], np.cumsum(C_lo)[:-1]])
    base_hi = np.concatenate([[0], np.cumsum(C_hi)[:-1]])
    n_lo, n_hi = int(C_lo.sum()), int(C_hi.sum())
    cons_base = np.concatenate([[0], np.cumsum(C_lo + C_hi)[:-1]])
    nch = n_lo + n_hi

    in_maps = []
    for k in range(N_CORES):
        c_k, s_k, dl_k, key, h_k = per_core[k]
        t_k = key >> 1
        grp_counts = np.bincount(key, minlength=NT * 2)
        grp_start = np.concatenate([[0], np.cumsum(grp_counts)[:-1]])
        rank = np.arange(len(key)) - grp_start[key]
        chunk_in_grp = rank >> 7
        lo_m = h_k == 0
        pos = np.where(lo_m, base_lo[t_k] * 128, base_hi[t_k] * 128) + rank

        idx_lo = np.zeros(n_lo * 128, dtype=np.int16)
        idx_hi = np.zeros(n_hi * 128, dtype=np.int16)
        idx_lo[pos[lo_m]] = c_k[lo_m].astype(np.int16)
        idx_hi[pos[~lo_m]] = (c_k[~lo_m] - NA).astype(np.int16)

        # consumption order per tile: lo chunks then hi chunks
        col = cons_base[t_k] + np.where(lo_m, chunk_in_grp,
                                        C_lo[t_k] + chunk_in_grp)
        row = rank & 127
        ncp = (nch + W - 1) // W * W
        meta_dst = np.full((128, ncp), 999.0, dtype=np.float16)
        meta_nrm = np.zeros((128, ncp), dtype=np.float16)
        meta_dst[row, col] = (dl_k & 127).astype(np.float16)
        meta_nrm[row, col] = (norm_src[s_k]
                              * norm_dst[dl_k + k * NPC]).astype(np.float16)

        in_maps.append(
            {
                "Xa": Xa,
                "Xb": Xb,
                "W1f": W1.astype(np.float16),
                "W2f": W2.astype(np.float16),
                "b1r": b1.reshape(1, 128).astype(np.float16),
                "b2r": b2.reshape(1, 128).astype(np.float16),
                "ones": np.ones((1, 128), dtype=np.float16),
                "iota8": np.tile(np.arange(128, dtype=np.float16), (128, W)),
                "idx_lo": np.tile(idx_lo.reshape(-1, 16).T, (8, 1)),
                "idx_hi": np.tile(idx_hi.reshape(-1, 16).T, (8, 1)),
                "meta_dst": meta_dst,
                "meta_nrm": meta_nrm,
            }
        )
    sched = (tuple(int(v) for v in C_lo), tuple(int(v) for v in C_hi))
    return in_maps, sched, nch


def _build_program(sched, nch):
    import concourse.bacc as bacc
    import concourse.mybir as mybir
    import concourse.tile as tile
    from concourse.library_config import mlp

    C_lo, C_hi = sched
    base_lo = np.concatenate([[0], np.cumsum(C_lo)[:-1]]).astype(int)
    base_hi = np.concatenate([[0], np.cumsum(C_hi)[:-1]]).astype(int)
    n_lo, n_hi = int(sum(C_lo)), int(sum(C_hi))
    cons_base = np.concatenate(
        [[0], np.cumsum(np.array(C_lo) + np.array(C_hi))[:-1]]
    ).astype(int)
    ncp = (nch + W - 1) // W * W
    f16 = mybir.dt.float16
    f32 = mybir.dt.float32
    AF = mybir.ActivationFunctionType
    ALU = mybir.AluOpType

    nc = bacc.Bacc("TRN2", target_bir_lowering=False, debug=False,
                   num_devices=N_CORES, num_swdge_queues=4)

    Xa_d = nc.dram_tensor("Xa", [NA, D], f16, kind="ExternalInput")
    Xb_d = nc.dram_tensor("Xb", [NB, D], f16, kind="ExternalInput")
    W1_d = nc.dram_tensor("W1f", [128, 128], f16, kind="ExternalInput")
    W2_d = nc.dram_tensor("W2f", [128, 128], f16, kind="ExternalInput")
    b1_d = nc.dram_tensor("b1r", [1, 128], f16, kind="ExternalInput")
    b2_d = nc.dram_tensor("b2r", [1, 128], f16, kind="ExternalInput")
    on_d = nc.dram_tensor("ones", [1, 128], f16, kind="ExternalInput")
    io_d = nc.dram_tensor("iota8", [128, W * 128], f16, kind="ExternalInput")
    ilo_d = nc.dram_tensor("idx_lo", [128, n_lo * 8], mybir.dt.int16,
                           kind="ExternalInput")
    ihi_d = nc.dram_tensor("idx_hi", [128, n_hi * 8], mybir.dt.int16,
                           kind="ExternalInput")
    mdst_d = nc.dram_tensor("meta_dst", [128, ncp], f16, kind="ExternalInput")
    mnrm_d = nc.dram_tensor("meta_nrm", [128, ncp], f16, kind="ExternalInput")

    rparts = (
        nc.dram_tensor("r_a", [RS, D], f16, kind="Internal"),
        nc.dram_tensor("r_b", [NPC - RS, D], f16, kind="Internal"),
    )
    Ra = nc.dram_tensor("Ra", [NA, D], f16, kind="Internal",
                        addr_space="Shared")
    Rb = nc.dram_tensor("Rb", [NB, D], f16, kind="Internal",
                        addr_space="Shared")
    out_d = nc.dram_tensor("outN", [NT * 128, D], f32, kind="ExternalOutput")

    qctr = [0]

    def next_q():
        q = qctr[0] % 4
        qctr[0] += 1
        return q

    with tile.TileContext(nc) as tc:
        with (
            tc.tile_pool(name="consts", bufs=1) as consts,
            tc.tile_pool(name="mt", bufs=24) as mt_pool,
            tc.tile_pool(name="st", bufs=16) as st_pool,
            tc.tile_pool(name="hb", bufs=6) as hb_pool,
            tc.tile_pool(name="pf", bufs=4) as pf_pool,
            tc.tile_pool(name="psa", bufs=3, space="PSUM") as psa_pool,
            tc.tile_pool(name="ps", bufs=5, space="PSUM") as ps_pool,
        ):
            nc.gpsimd.load_library(mlp)

            W1f = consts.tile([128, 128], f16, tag="W1f")
            W2f = consts.tile([128, 128], f16, tag="W2f")
            b1r = consts.tile([1, 128], f16, tag="b1r")
            b2r = consts.tile([1, 128], f16, tag="b2r")
            ones = consts.tile([1, 128], f16, tag="ones")
            iota8 = consts.tile([128, W, 128], f16, tag="iota8")
            idx_lo = consts.tile([128, n_lo * 8], mybir.dt.int16, tag="ilo")
            idx_hi = consts.tile([128, n_hi * 8], mybir.dt.int16, tag="ihi")
            mdst = consts.tile([128, ncp], f16, tag="mdst")
            mnrm = consts.tile([128, ncp], f16, tag="mnrm")
            nc.sync.dma_start(idx_lo[:], ilo_d.ap())
            nc.sync.dma_start(mdst[:], mdst_d.ap())
            nc.sync.dma_start(mnrm[:], mnrm_d.ap())
            nc.sync.dma_start(idx_hi[:], ihi_d.ap())
            nc.sync.dma_start(W1f[:], W1_d.ap())
            nc.sync.dma_start(W2f[:], W2_d.ap())
            nc.sync.dma_start(b1r[:], b1_d.ap())
            nc.sync.dma_start(b2r[:], b2_d.ap())
            nc.sync.dma_start(ones[:], on_d.ap())
            nc.sync.dma_start(iota8[:],
                              io_d.ap().rearrange("p (a e) -> p a e", a=W))

            BT = 4
            TSPLIT = RS // 128      # 32

            def make_h_writer(h_dram, t_lo, t_hi):
                """Write tiles [t_lo, t_hi) into h_dram (rows rebased)."""
                nfull = min(t_hi, NPC // 128) - t_lo
                h3 = h_dram.ap()[0 : nfull * 128, :].rearrange(
                    "(a p) d -> p a d", p=128
                )
                state = {}

                def write(t, produce):
                    tl_ = t - t_lo
                    if tl_ < nfull:
                        g = tl_ - tl_ % BT
                        if tl_ % BT == 0:
                            state["buf"] = hb_pool.tile(
                                [128, BT, 128], f16, tag="hstage", name="hstage"
                            )
                        produce(state["buf"][:, tl_ % BT, :])
                        if tl_ % BT == BT - 1 or tl_ == nfull - 1:
                            n = tl_ - g + 1
                            nc.scalar.dma_start(h3[:, g : g + n, :],
                                                state["buf"][:, 0:n, :])
                    else:
                        rows = NPC - t * 128
                        tl = hb_pool.tile([128, 128], f16, tag="hrag",
                                          name="hrag")
                        produce(tl[:])
                        nc.scalar.dma_start(
                            h_dram.ap()[tl_ * 128 : tl_ * 128 + rows, :],
                            tl[:rows, :],
                        )

                return write

            def sub_allgather(h_sub, H_out):
                nc.gpsimd.collective_compute(
                    "AllGather", mybir.AluOpType.bypass,
                    replica_groups=[list(range(N_CORES))],
                    ins=[h_sub.ap()], outs=[H_out.ap()],
                )

            def agg_phase(Ta, Tb, out_cb, pre_loop=None):
                """Gather in fixed W-chunk windows per (lo, hi) stream; per
                dst tile accumulate segment-sum matmuls into psum [f x d],
                then out_cb(t, pa). Selector built on DVE per S-window."""
                streams = {"lo": (Ta, idx_lo, n_lo), "hi": (Tb, idx_hi, n_hi)}
                mt_tiles = {}
                st_tiles = {}

                def ensure_window(s, w):
                    key = (s, w)
                    if key in mt_tiles:
                        return
                    tab, idx_t, n_s = streams[s]
                    cb = w * W
                    cw = min(W, n_s - cb)
                    mt = mt_pool.tile([128, cw, 128], f16, tag="mt")
                    nc.gpsimd.dma_gather(
                        mt[:], tab.ap(),
                        idx_t[:, cb * 8 : (cb + cw) * 8],
                        cw * 128, cw * 128, 128,
                        queue_num=next_q(),
                    )
                    mt_tiles[key] = mt

                def ensure_s(cons):
                    sw, so = cons // W, cons % W
                    if sw not in st_tiles:
                        st = st_pool.tile([128, W, 128], f16, tag="st")
                        md = mdst[:, sw * W : (sw + 1) * W].rearrange(
                            "p (a b) -> p a b", b=1).to_broadcast([128, W, 128])
                        mn = mnrm[:, sw * W : (sw + 1) * W].rearrange(
                            "p (a b) -> p a b", b=1).to_broadcast([128, W, 128])
                        nc.vector.tensor_tensor(out=st[:], in0=iota8[:],
                                                in1=md, op=ALU.is_equal)
                        nc.vector.tensor_tensor(out=st[:], in0=st[:],
                                                in1=mn, op=ALU.mult)
                        st_tiles[sw] = st
                    return st_tiles[sw][:, so, :]

                # Prefetch lo windows so the in-order Pool engine has queued
                # DMA work to drain while the first hi window waits on the
                # part-b table.
                for w in range(min(20, (n_lo + W - 1) // W)):
                    ensure_window("lo", w)
                if pre_loop is not None:
                    pre_loop()

                def chunk_list(t):
                    out = []
                    for i in range(C_lo[t] + C_hi[t]):
                        if i < C_lo[t]:
                            out.append(("lo", int(base_lo[t]) + i))
                        else:
                            out.append(("hi", int(base_hi[t]) + (i - C_lo[t])))
                    return out

                for t in range(NT):
                    # Emit tile t+2's gather windows and S-builds ahead so
                    # that by the time their matmuls issue, every semaphore
                    # is already satisfied (fast-path dispatch on the PE).
                    if t + 2 < NT:
                        for i2, (s2, sc2) in enumerate(chunk_list(t + 2)):
                            ensure_window(s2, sc2 // W)
                            ensure_s(int(cons_base[t + 2]) + i2)
                    chunks = chunk_list(t)
                    pa = ps_pool.tile([128, 128], f32, tag="pa")
                    for i, (s, sc) in enumerate(chunks):
                        w, o = sc // W, sc % W
                        ensure_window(s, w)
                        cons = int(cons_base[t]) + i
                        s_ap = ensure_s(cons)
                        nc.tensor.matmul(
                            pa[:], mt_tiles[(s, w)][:, o, :], s_ap,
                            start=(i == 0), stop=(i == len(chunks) - 1),
                        )
                    out_cb(t, pa)

            # ---- layer 1: agg(x) -> @W1 + b1 -> relu -> r parts + AG ----
            w1_a = make_h_writer(rparts[0], 0, TSPLIT)
            w1_b = make_h_writer(rparts[1], TSPLIT, NT)

            def phase_1(t, pa):
                paf = pf_pool.tile([128, 128], f16, tag="paf")
                nc.scalar.activation(paf[:], pa[:], AF.Copy)
                ph = psa_pool.tile([128, 128], f32, tag="ph", name="ph")
                nc.tensor.matmul(ph[:], paf[:], W1f[:], start=True, stop=False)
                nc.tensor.matmul(ph[:], ones[:], b1r[:], start=False, stop=True)
                (w1_a if t < TSPLIT else w1_b)(
                    t, lambda dst, ph=ph: nc.scalar.activation(
                        dst, ph[:], AF.Relu))
                # Trigger the part-a AllGather a few tiles past TSPLIT so the
                # Pool engine (which runs ~8 windows ahead of consumption)
                # reaches the blocking trigger only after the h-writes it
                # waits on have completed.
                if t == TSPLIT + 6:
                    sub_allgather(rparts[0], Ra)

            agg_phase(Xa_d, Xb_d, phase_1)

            # ---- layer 2: agg(r) -> @W2 + b2 -> out (node-major) ----
            o3 = out_d.ap().rearrange("(a p) d -> p a d", p=128)
            ostate = {}

            def phase_2(t, pa):
                paf = pf_pool.tile([128, 128], f16, tag="paf")
                nc.scalar.activation(paf[:], pa[:], AF.Copy)
                ph = psa_pool.tile([128, 128], f32, tag="ph", name="ph2")
                nc.tensor.matmul(ph[:], paf[:], W2f[:], start=True, stop=False)
                nc.tensor.matmul(ph[:], ones[:], b2r[:], start=False, stop=True)
                g = t - t % BT
                if t % BT == 0:
                    ostate["buf"] = hb_pool.tile([128, BT, 128], f32,
                                                 tag="ostage", name="ostage")
                nc.scalar.activation(ostate["buf"][:, t % BT, :], ph[:],
                                     AF.Copy)
                if t % BT == BT - 1 or t == NT - 1:
                    n = t - g + 1
                    nc.scalar.dma_start(o3[:, g : g + n, :],
                                        ostate["buf"][:, 0:n, :])

            # The part-b AllGather trigger is emitted after layer 2's lo
            # prefetch so those window issues queue ahead of the blocking
            # trigger on the in-order Pool engine.
            agg_phase(Ra, Rb, phase_2,
                      pre_loop=lambda: sub_allgather(rparts[1], Rb))

    nc.compile()
    return nc


def kernel(x, src, dst, W1, b1, W2, b2):
    from concourse.bass_utils import run_bass_kernel_spmd

    in_maps, sched, nch = _host_prep(x, src, dst, W1, b1, W2, b2)
    key = (sched, nch)
    if key not in _CACHE:
        _CACHE[key] = _build_program(sched, nch)
    nc = _CACHE[key]
    res = run_bass_kernel_spmd(nc, in_maps, core_ids=list(range(N_CORES)))
    out = np.empty((N_NODES, D), dtype=np.float32)
    for k in range(N_CORES):
        out[k * NPC : (k + 1) * NPC] = res.results[k]["outN"][:NPC]
    return out


# revision 53
# speedup vs baseline: 1.1563x; 1.0289x over previous
"""Two-layer GCN (GraphConv norm='both') on 8 Trainium2 NeuronCores.

Strategy (graph/data parallel per the sharding hint):
  - dst nodes partitioned across 8 cores (6250 rows each); weights replicated.
  - The dense weight is COMMUTED through the segment-sum:
    sum_e ns[s]*nd[d]*(x[s] @ W) = (sum_e ns[s]*nd[d]*x[s]) @ W, so layer 1
    aggregates RAW x rows. Each core receives the full x as a plain input
    split into two HBM tables (no phase-A compute, no layer-1 collective:
    gathers start immediately). Layer 2 aggregates relu rows r = relu(agg1
    @ W1 + b1) and applies W2 + b2 after its segment-sum; only the r table
    needs the halo-exchange AllGather (one per layer boundary).
  - Edge aggregation per core: edges grouped by (dst-tile, table-half) and
    sorted by source; source rows fetched with SWDGE dma_gather (256B f16
    rows, 8-chunk single-packet windows rotating over 4 SWDGE queues);
    segment-sum on the PE as one-hot matmuls into PSUM [f_in x dst]. The
    edge-weight selector S (carrying ns*nd) is built ON-CHIP on the DVE:
    S = (iota == dstcol) * nrm from a tiny per-chunk metadata table.
  - int16 gather indices cover only 32768 rows, so tables are split at row
    32768 (part a = each core's rows 0..4095 concatenated, part b = the
    rest). 256B single-row gathers have zero fetch waste, and part-a
    gathers depend only on the part-a AllGather (Tile tracks DRAM deps per
    tensor), letting next-layer gathers start while part b is in flight.
  - Biases are rank-1 matmul updates (ones-column x bias-row) into the
    same PSUM tile as the dense matmul; per-tile flow is
    pa=[f,d] -> f16 -> @W -> [d,f] node-major (+bias) -> relu/copy.

All feature math runs on-device in fp16 (f32 PSUM accumulation); index
preprocessing (edge partitioning/sorting, degree counts, gather schedule)
is host-side sharding prep.
"""

import numpy as np

N_NODES = 50000
N_EDGES = 600000
D = 128
N_CORES = 8
NPC = N_NODES // N_CORES          # 6250 nodes per core
NT = (NPC + 127) // 128           # 49 dst tiles per core
RS = 4096                         # part-a rows per core (8*RS = 32768)
NA = N_CORES * RS                 # 32768 rows in table a
NB = N_NODES - NA                 # 17232 rows in table b
W = 8                             # gather window size (chunks per call)

_CACHE = {}


def _host_prep(x, src, dst, W1, b1, W2, b2):
    x = np.asarray(x, dtype=np.float32)
    src = np.asarray(src, dtype=np.int64)
    dst = np.asarray(dst, dtype=np.int64)
    W1 = np.asarray(W1, dtype=np.float32)
    W2 = np.asarray(W2, dtype=np.float32)
    b1 = np.asarray(b1, dtype=np.float32)
    b2 = np.asarray(b2, dtype=np.float32)

    deg_out = np.bincount(src, minlength=N_NODES).astype(np.float32)
    deg_in = np.bincount(dst, minlength=N_NODES).astype(np.float32)
    norm_src = np.where(deg_out > 0, 1.0 / np.sqrt(np.maximum(deg_out, 1.0)), 0.0)
    norm_dst = np.where(deg_in > 0, 1.0 / np.sqrt(np.maximum(deg_in, 1.0)), 0.0)
    norm_src = norm_src.astype(np.float32)
    norm_dst = norm_dst.astype(np.float32)

    # Map node -> row in the split-table layout: part a = each core's rows
    # [0, RS) concatenated; part b = rows [RS, NPC) concatenated.
    nodes = np.arange(N_NODES, dtype=np.int64)
    ksn, rsn = nodes // NPC, nodes % NPC
    cat_of_node = np.where(rsn < RS, ksn * RS + rsn,
                           NA + ksn * (NPC - RS) + (rsn - RS))
    xcat = np.empty((N_NODES, D), dtype=np.float16)
    xcat[cat_of_node] = x.astype(np.float16)
    Xa, Xb = np.ascontiguousarray(xcat[:NA]), np.ascontiguousarray(xcat[NA:])

    cat = cat_of_node[src]
    half = (cat >= NA).astype(np.int64)

    # --- per-core edge grouping by (dst tile, half), sorted by cat ---
    per_core = []
    cnts = np.zeros((N_CORES, NT, 2), dtype=np.int64)
    for k in range(N_CORES):
        m = (dst >= k * NPC) & (dst < (k + 1) * NPC)
        c_k = cat[m]
        s_k = src[m]
        dl_k = dst[m] - k * NPC
        t_k = dl_k >> 7
        h_k = half[m]
        key = t_k * 2 + h_k
        order = np.lexsort((c_k, key))
        per_core.append((c_k[order], s_k[order], dl_k[order], key[order],
                         h_k[order]))
        cnts[k] = np.bincount(key, minlength=NT * 2).reshape(NT, 2)

    # shared static schedule: chunks per (tile, half), max over cores
    C_lo = np.maximum.reduce([(cnts[k, :, 0] + 127) // 128 for k in range(N_CORES)])
    C_hi = np.maximum.reduce([(cnts[k, :, 1] + 127) // 128 for k in range(N_CORES)])
    C_lo = np.where((C_lo + C_hi) == 0, 1, C_lo)
    base_lo = np.concatenate([[0], np.cumsum(C_lo)[:-1]])
    base_hi = np.concatenate([[0], np.cumsum(C_hi)[:-1]])
    n_lo, n_hi = int(C_lo.sum()), int(C_hi.sum())
    cons_base = np.concatenate([[0], np.cumsum(C_lo + C_hi)[:-1]])
    nch = n_lo + n_hi

    in_maps = []
    for k in range(N_CORES):
        c_k, s_k, dl_k, key, h_k = per_core[k]
        t_k = key >> 1
        grp_counts = np.bincount(key, minlength=NT * 2)
        grp_start = np.concatenate([[0], np.cumsum(grp_counts)[:-1]])
        rank = np.arange(len(key)) - grp_start[key]
        chunk_in_grp = rank >> 7
        lo_m = h_k == 0
        pos = np.where(lo_m, base_lo[t_k] * 128, base_hi[t_k] * 128) + rank

        idx_lo = np.zeros(n_lo * 128, dtype=np.int16)
        idx_hi = np.zeros(n_hi * 128, dtype=np.int16)
        idx_lo[pos[lo_m]] = c_k[lo_m].astype(np.int16)
        idx_hi[pos[~lo_m]] = (c_k[~lo_m] - NA).astype(np.int16)

        # consumption order per tile: lo chunks then hi chunks
        col = cons_base[t_k] + np.where(lo_m, chunk_in_grp,
                                        C_lo[t_k] + chunk_in_grp)
        row = rank & 127
        ncp = (nch + W - 1) // W * W
        meta_dst = np.full((128, ncp), 999.0, dtype=np.float16)
        meta_nrm = np.zeros((128, ncp), dtype=np.float16)
        meta_dst[row, col] = (dl_k & 127).astype(np.float16)
        meta_nrm[row, col] = (norm_src[s_k]
                              * norm_dst[dl_k + k * NPC]).astype(np.float16)

        in_maps.append(
            {
                "Xa": Xa,
                "Xb": Xb,
                "W1f": W1.astype(np.float16),
                "W2f": W2.astype(np.float16),
                "b1r": b1.reshape(1, 128).astype(np.float16),
                "b2r": b2.reshape(1, 128).astype(np.float16),
                "ones": np.ones((1, 128), dtype=np.float16),
                "iota8": np.tile(np.arange(128, dtype=np.float16), (128, W)),
                "idx_lo": np.tile(idx_lo.reshape(-1, 16).T, (8, 1)),
                "idx_hi": np.tile(idx_hi.reshape(-1, 16).T, (8, 1)),
                "meta_dst": meta_dst,
                "meta_nrm": meta_nrm,
            }
        )
    sched = (tuple(int(v) for v in C_lo), tuple(int(v) for v in C_hi))
    return in_maps, sched, nch


def _build_program(sched, nch):
    import concourse.bacc as bacc
    import concourse.mybir as mybir
    import concourse.tile as tile
    from concourse.library_config import mlp

    C_lo, C_hi = sched
    base_lo = np.concatenate([[0], np.cumsum(C_lo)[:-1]]).astype(int)
    base_hi = np.concatenate([[0], np.cumsum(C_hi)[:-1]]).astype(int)
    n_lo, n_hi = int(sum(C_lo)), int(sum(C_hi))
    cons_base = np.concatenate(
        [[0], np.cumsum(np.array(C_lo) + np.array(C_hi))[:-1]]
    ).astype(int)
    ncp = (nch + W - 1) // W * W
    f16 = mybir.dt.float16
    f32 = mybir.dt.float32
    AF = mybir.ActivationFunctionType
    ALU = mybir.AluOpType

    nc = bacc.Bacc("TRN2", target_bir_lowering=False, debug=False,
                   num_devices=N_CORES, num_swdge_queues=4)

    Xa_d = nc.dram_tensor("Xa", [NA, D], f16, kind="ExternalInput")
    Xb_d = nc.dram_tensor("Xb", [NB, D], f16, kind="ExternalInput")
    W1_d = nc.dram_tensor("W1f", [128, 128], f16, kind="ExternalInput")
    W2_d = nc.dram_tensor("W2f", [128, 128], f16, kind="ExternalInput")
    b1_d = nc.dram_tensor("b1r", [1, 128], f16, kind="ExternalInput")
    b2_d = nc.dram_tensor("b2r", [1, 128], f16, kind="ExternalInput")
    on_d = nc.dram_tensor("ones", [1, 128], f16, kind="ExternalInput")
    io_d = nc.dram_tensor("iota8", [128, W * 128], f16, kind="ExternalInput")
    ilo_d = nc.dram_tensor("idx_lo", [128, n_lo * 8], mybir.dt.int16,
                           kind="ExternalInput")
    ihi_d = nc.dram_tensor("idx_hi", [128, n_hi * 8], mybir.dt.int16,
                           kind="ExternalInput")
    mdst_d = nc.dram_tensor("meta_dst", [128, ncp], f16, kind="ExternalInput")
    mnrm_d = nc.dram_tensor("meta_nrm", [128, ncp], f16, kind="ExternalInput")

    rparts = (
        nc.dram_tensor("r_a", [RS, D], f16, kind="Internal"),
        nc.dram_tensor("r_b", [NPC - RS, D], f16, kind="Internal"),
    )
    Ra = nc.dram_tensor("Ra", [NA, D], f16, kind="Internal",
                        addr_space="Shared")
    Rb = nc.dram_tensor("Rb", [NB, D], f16, kind="Internal",
                        addr_space="Shared")
    out_d = nc.dram_tensor("outN", [NT * 128, D], f32, kind="ExternalOutput")

    qctr = [0]

    def next_q():
        q = qctr[0] % 4
        qctr[0] += 1
        return q

    with tile.TileContext(nc) as tc:
        with (
            tc.tile_pool(name="consts", bufs=1) as consts,
            tc.tile_pool(name="mt", bufs=24) as mt_pool,
            tc.tile_pool(name="st", bufs=16) as st_pool,
            tc.tile_pool(name="hb", bufs=6) as hb_pool,
            tc.tile_pool(name="pf", bufs=4) as pf_pool,
            tc.tile_pool(name="psa", bufs=3, space="PSUM") as psa_pool,
            tc.tile_pool(name="ps", bufs=5, space="PSUM") as ps_pool,
        ):
            nc.gpsimd.load_library(mlp)

            W1f = consts.tile([128, 128], f16, tag="W1f")
            W2f = consts.tile([128, 128], f16, tag="W2f")
            b1r = consts.tile([1, 128], f16, tag="b1r")
            b2r = consts.tile([1, 128], f16, tag="b2r")
            ones = consts.tile([1, 128], f16, tag="ones")
            iota8 = consts.tile([128, W, 128], f16, tag="iota8")
            idx_lo = consts.tile([128, n_lo * 8], mybir.dt.int16, tag="ilo")
            idx_hi = consts.tile([128, n_hi * 8], mybir.dt.int16, tag="ihi")
            mdst = consts.tile([128, ncp], f16, tag="mdst")
            mnrm = consts.tile([128, ncp], f16, tag="mnrm")
            nc.sync.dma_start(idx_lo[:], ilo_d.ap())
            nc.sync.dma_start(mdst[:], mdst_d.ap())
            nc.sync.dma_start(mnrm[:], mnrm_d.ap())
            nc.sync.dma_start(idx_hi[:], ihi_d.ap())
            nc.sync.dma_start(W1f[:], W1_d.ap())
            nc.sync.dma_start(W2f[:], W2_d.ap())
            nc.sync.dma_start(b1r[:], b1_d.ap())
            nc.sync.dma_start(b2r[:], b2_d.ap())
            nc.sync.dma_start(ones[:], on_d.ap())
            nc.sync.dma_start(iota8[:],
                              io_d.ap().rearrange("p (a e) -> p a e", a=W))

            BT = 4
            TSPLIT = RS // 128      # 32

            def make_h_writer(h_dram, t_lo, t_hi):
                """Write tiles [t_lo, t_hi) into h_dram (rows rebased)."""
                nfull = min(t_hi, NPC // 128) - t_lo
                h3 = h_dram.ap()[0 : nfull * 128, :].rearrange(
                    "(a p) d -> p a d", p=128
                )
                state = {}

                def write(t, produce):
                    tl_ = t - t_lo
                    if tl_ < nfull:
                        g = tl_ - tl_ % BT
                        if tl_ % BT == 0:
                            state["buf"] = hb_pool.tile(
                                [128, BT, 128], f16, tag="hstage", name="hstage"
                            )
                        produce(state["buf"][:, tl_ % BT, :])
                        if tl_ % BT == BT - 1 or tl_ == nfull - 1:
                            n = tl_ - g + 1
                            nc.scalar.dma_start(h3[:, g : g + n, :],
                                                state["buf"][:, 0:n, :])
                    else:
                        rows = NPC - t * 128
                        tl = hb_pool.tile([128, 128], f16, tag="hrag",
                                          name="hrag")
                        produce(tl[:])
                        nc.scalar.dma_start(
                            h_dram.ap()[tl_ * 128 : tl_ * 128 + rows, :],
                            tl[:rows, :],
                        )

                return write

            def sub_allgather(h_sub, H_out):
                nc.gpsimd.collective_compute(
                    "AllGather", mybir.AluOpType.bypass,
                    replica_groups=[list(range(N_CORES))],
                    ins=[h_sub.ap()], outs=[H_out.ap()],
                )

            def agg_phase(Ta, Tb, out_cb, pre_loop=None):
                """Gather in fixed W-chunk windows per (lo, hi) stream; per
                dst tile accumulate segment-sum matmuls into psum [f x d],
                then out_cb(t, pa). Selector built on DVE per S-window."""
                streams = {"lo": (Ta, idx_lo, n_lo), "hi": (Tb, idx_hi, n_hi)}
                mt_tiles = {}
                st_tiles = {}

                def ensure_window(s, w):
                    key = (s, w)
                    if key in mt_tiles:
                        return
                    tab, idx_t, n_s = streams[s]
                    cb = w * W
                    cw = min(W, n_s - cb)
                    mt = mt_pool.tile([128, cw, 128], f16, tag="mt")
                    nc.gpsimd.dma_gather(
                        mt[:], tab.ap(),
                        idx_t[:, cb * 8 : (cb + cw) * 8],
                        cw * 128, cw * 128, 128,
                        queue_num=next_q(),
                    )
                    mt_tiles[key] = mt

                def ensure_s(cons):
                    sw, so = cons // W, cons % W
                    if sw not in st_tiles:
                        st = st_pool.tile([128, W, 128], f16, tag="st")
                        md = mdst[:, sw * W : (sw + 1) * W].rearrange(
                            "p (a b) -> p a b", b=1).to_broadcast([128, W, 128])
                        mn = mnrm[:, sw * W : (sw + 1) * W].rearrange(
                            "p (a b) -> p a b", b=1).to_broadcast([128, W, 128])
                        nc.vector.tensor_tensor(out=st[:], in0=iota8[:],
                                                in1=md, op=ALU.is_equal)
                        nc.vector.tensor_tensor(out=st[:], in0=st[:],
                                                in1=mn, op=ALU.mult)
                        st_tiles[sw] = st
                    return st_tiles[sw][:, so, :]

                # Prefetch lo windows so the in-order Pool engine has queued
                # DMA work to drain while the first hi window waits on the
                # part-b table.
                for w in range(min(20, (n_lo + W - 1) // W)):
                    ensure_window("lo", w)
                if pre_loop is not None:
                    pre_loop()

                def chunk_list(t):
                    out = []
                    for i in range(C_lo[t] + C_hi[t]):
                        if i < C_lo[t]:
                            out.append(("lo", int(base_lo[t]) + i))
                        else:
                            out.append(("hi", int(base_hi[t]) + (i - C_lo[t])))
                    return out

                pending = None
                for t in range(NT):
                    # Emit tile t+2's gather windows and S-builds ahead so
                    # that by the time their matmuls issue, every semaphore
                    # is already satisfied (fast-path dispatch on the PE).
                    if t + 2 < NT:
                        for i2, (s2, sc2) in enumerate(chunk_list(t + 2)):
                            ensure_window(s2, sc2 // W)
                            ensure_s(int(cons_base[t + 2]) + i2)
                    chunks = chunk_list(t)
                    pa = ps_pool.tile([128, 128], f32, tag="pa")
                    for i, (s, sc) in enumerate(chunks):
                        w, o = sc // W, sc % W
                        ensure_window(s, w)
                        cons = int(cons_base[t]) + i
                        s_ap = ensure_s(cons)
                        nc.tensor.matmul(
                            pa[:], mt_tiles[(s, w)][:, o, :], s_ap,
                            start=(i == 0), stop=(i == len(chunks) - 1),
                        )
                    # Defer the per-tile chain one tile: its cross-engine
                    # round-trips (psum copy -> W matmul -> relu/out) then
                    # overlap tile t+1's chunk matmuls instead of stalling
                    # the in-order PE queue.
                    if pending is not None:
                        out_cb(pending[0], pending[1])
                    pending = (t, pa)
                out_cb(pending[0], pending[1])

            # ---- layer 1: agg(x) -> @W1 + b1 -> relu -> r parts + AG ----
            w1_a = make_h_writer(rparts[0], 0, TSPLIT)
            w1_b = make_h_writer(rparts[1], TSPLIT, NT)

            def phase_1(t, pa):
                paf = pf_pool.tile([128, 128], f16, tag="paf")
                nc.scalar.activation(paf[:], pa[:], AF.Copy)
                ph = psa_pool.tile([128, 128], f32, tag="ph", name="ph")
                nc.tensor.matmul(ph[:], paf[:], W1f[:], start=True, stop=False)
                nc.tensor.matmul(ph[:], ones[:], b1r[:], start=False, stop=True)
                (w1_a if t < TSPLIT else w1_b)(
                    t, lambda dst, ph=ph: nc.scalar.activation(
                        dst, ph[:], AF.Relu))
                # Trigger the part-a AllGather a few tiles past TSPLIT so the
                # Pool engine (which runs ~8 windows ahead of consumption)
                # reaches the blocking trigger only after the h-writes it
                # waits on have completed.
                if t == TSPLIT + 6:
                    sub_allgather(rparts[0], Ra)

            agg_phase(Xa_d, Xb_d, phase_1)

            # ---- layer 2: agg(r) -> @W2 + b2 -> out (node-major) ----
            o3 = out_d.ap().rearrange("(a p) d -> p a d", p=128)
            ostate = {}

            def phase_2(t, pa):
                paf = pf_pool.tile([128, 128], f16, tag="paf")
                nc.scalar.activation(paf[:], pa[:], AF.Copy)
                ph = psa_pool.tile([128, 128], f32, tag="ph", name="ph2")
                nc.tensor.matmul(ph[:], paf[:], W2f[:], start=True, stop=False)
                nc.tensor.matmul(ph[:], ones[:], b2r[:], start=False, stop=True)
                g = t - t % BT
                if t % BT == 0:
                    ostate["buf"] = hb_pool.tile([128, BT, 128], f32,
                                                 tag="ostage", name="ostage")
                nc.scalar.activation(ostate["buf"][:, t % BT, :], ph[:],
                                     AF.Copy)
                if t % BT == BT - 1 or t == NT - 1:
                    n = t - g + 1
                    nc.scalar.dma_start(o3[:, g : g + n, :],
                                        ostate["buf"][:, 0:n, :])

            # The part-b AllGather trigger is emitted after layer 2's lo
            # prefetch so those window issues queue ahead of the blocking
            # trigger on the in-order Pool engine.
            agg_phase(Ra, Rb, phase_2,
                      pre_loop=lambda: sub_allgather(rparts[1], Rb))

    nc.compile()
    return nc


def kernel(x, src, dst, W1, b1, W2, b2):
    from concourse.bass_utils import run_bass_kernel_spmd

    in_maps, sched, nch = _host_prep(x, src, dst, W1, b1, W2, b2)
    key = (sched, nch)
    if key not in _CACHE:
        _CACHE[key] = _build_program(sched, nch)
    nc = _CACHE[key]
    res = run_bass_kernel_spmd(nc, in_maps, core_ids=list(range(N_CORES)))
    out = np.empty((N_NODES, D), dtype=np.float32)
    for k in range(N_CORES):
        out[k * NPC : (k + 1) * NPC] = res.results[k]["outN"][:NPC]
    return out
